# revision 7
# baseline (speedup 1.0000x reference)
"""Distributed Trainium2 (Bass/Tile) kernel for a pre-LN transformer block.

Reference computation (per batch element):
    xn = LN1(x); q,k,v = per-head projections of xn
    attn = causal-softmax(q k^T / sqrt(dh)) v
    x1 = x + concat_heads(attn) @ w_proj + b_proj
    out = x1 + relu(LN2(x1) @ w1 + b1) @ w2 + b2

Sharding over 8 NeuronCores: core c handles batch b=c//4 and head group
g=c%4 (4 of 16 heads).  Attention + projection partials are head-parallel;
one ReduceScatter(add) over each 4-core group turns projection partials
into per-core 512-row slices of x1; the FFN then runs sequence-parallel
(512 rows per core) with no further communication.  The host assembles the
8 [512,1024] outputs into the full [2,2048,1024] result.

All matmuls run as float32r (TF32-like) with fp32 PSUM accumulation.
"""

import numpy as np

import concourse.bass as bass
import concourse.mybir as mybir
import concourse.tile as tile
from contextlib import ExitStack
from concourse import bacc
from concourse.masks import make_identity
from concourse.bass_utils import run_bass_kernel_spmd

T = 2048          # sequence length
D = 1024          # embedding dim
H = 16            # total heads
DH = 64           # head dim
HL = 4            # heads per core
TG = 512          # rows per core in the FFN phase
DF = 4096         # FFN hidden dim
EPS = 1e-5
N_CORES = 8

f32 = mybir.dt.float32
f32r = mybir.dt.float32r
AF = mybir.ActivationFunctionType
ALU = mybir.AluOpType


def _ln_stats(nc, pool, xt, width, eps_ap):
    """Per-partition mean/var over `width` free elements -> (rstd, neg_mu_rstd)."""
    nchunk = width // 512
    bns = pool.tile([128, nchunk, 6], f32, tag="bns")
    for i in range(nchunk):
        nc.vector.bn_stats(bns[:, i, :], xt[:, i * 512:(i + 1) * 512])
    agg = pool.tile([128, 2], f32, tag="agg")
    nc.vector.bn_aggr(agg[:], bns[:].rearrange("p a b -> p (a b)"))
    std = pool.tile([128, 1], f32, tag="std")
    nc.scalar.activation(std[:], agg[:, 1:2], AF.Sqrt, bias=eps_ap)
    rstd = pool.tile([128, 1], f32, tag="rstd")
    nc.vector.reciprocal(rstd[:], std[:])
    nmr = pool.tile([128, 1], f32, tag="nmr")
    nc.vector.tensor_scalar(nmr[:], agg[:, 0:1], rstd[:], -1.0, ALU.mult, ALU.mult)
    return rstd, nmr


def _build():
    nc = bacc.Bacc("TRN2", target_bir_lowering=False, debug=False,
                   num_devices=N_CORES)

    x_in = nc.dram_tensor("x", [T, D], f32, kind="ExternalInput")
    wq_in = nc.dram_tensor("wq", [128, 8, HL * DH], f32, kind="ExternalInput")
    wk_in = nc.dram_tensor("wk", [128, 8, HL * DH], f32, kind="ExternalInput")
    wv_in = nc.dram_tensor("wv", [128, 8, HL * DH], f32, kind="ExternalInput")
    wp_in = nc.dram_tensor("wp", [128, 2, D], f32, kind="ExternalInput")
    bp_in = nc.dram_tensor("bp", [1, D], f32, kind="ExternalInput")
    w1_in = nc.dram_tensor("w1", [128, 32, 8, 128], f32, kind="ExternalInput")
    b1_in = nc.dram_tensor("b1", [128, 32], f32, kind="ExternalInput")
    w2_in = nc.dram_tensor("w2", [DF, D], f32, kind="ExternalInput")
    b2_in = nc.dram_tensor("b2", [1, D], f32, kind="ExternalInput")
    g1_in = nc.dram_tensor("ln1g", [128, 8], f32, kind="ExternalInput")
    be1_in = nc.dram_tensor("ln1b", [128, 8], f32, kind="ExternalInput")
    g2_in = nc.dram_tensor("ln2g", [128, 8], f32, kind="ExternalInput")
    be2_in = nc.dram_tensor("ln2b", [128, 8], f32, kind="ExternalInput")
    xg_in = nc.dram_tensor("xg", [TG, D], f32, kind="ExternalInput")
    out_dram = nc.dram_tensor("out", [TG, D], f32, kind="ExternalOutput")

    with tile.TileContext(nc) as tc, ExitStack() as top:
        persist = top.enter_context(tc.tile_pool(name="persist", bufs=1))
        mid = top.enter_context(tc.tile_pool(name="mid", bufs=4))
        consts = top.enter_context(tc.tile_pool(name="consts", bufs=1))
        dram = top.enter_context(tc.tile_pool(name="dram", bufs=1, space="DRAM"))

        # ---- constants ----
        ident = consts.tile([128, 128], f32, tag="ident")
        make_identity(nc, ident[:])
        masks = consts.tile([128, HL, 512], f32, tag="masks")
        nc.vector.memset(masks[:], 1.0)
        for d in range(HL):
            # keep (1.0) where global_t - global_s >= 0, i.e. f - p - 128*d >= 0
            nc.gpsimd.affine_select(
                out=masks[:, d, :], in_=masks[:, d, :],
                compare_op=ALU.is_ge, fill=0.0,
                base=-128 * d, pattern=[[1, 512]], channel_multiplier=-1)
        g1t = consts.tile([128, 8], f32, tag="g1t")
        nc.sync.dma_start(g1t[:], g1_in[:])
        be1t = consts.tile([128, 8], f32, tag="be1t")
        nc.sync.dma_start(be1t[:], be1_in[:])
        g2t = consts.tile([128, 8], f32, tag="g2t")
        nc.sync.dma_start(g2t[:], g2_in[:])
        be2t = consts.tile([128, 8], f32, tag="be2t")
        nc.sync.dma_start(be2t[:], be2_in[:])
        epst = consts.tile([128, 1], f32, tag="epst")
        nc.vector.memset(epst[:], EPS)
        b1s = consts.tile([128, 32], f32, tag="b1s")
        nc.sync.dma_start(b1s[:], b1_in[:])
        wp_sb = consts.tile([128, 2, D], f32r, tag="wp_sb")
        nc.sync.dma_start(wp_sb[:], wp_in[:].bitcast(f32r))

        # ---- persistent activation tiles ----
        xnT = persist.tile([128, 8, T], f32r, tag="big")        # LN1(x)^T
        qT = mid.tile([128, 2, T], f32r, tag="mid")             # q^T (4 heads)
        kT = mid.tile([128, 2, T], f32r, tag="mid")             # k^T
        v_sb = mid.tile([128, 16, HL, DH + 1], f32r, tag="mid")  # v + ones col
        oaT = mid.tile([128, 2, T], f32r, tag="mid")            # attn out^T

        # ================= Phase 1: LN1 + transpose =================
        with ExitStack() as ph:
            work = ph.enter_context(tc.tile_pool(name="p1work", bufs=3))
            small = ph.enter_context(tc.tile_pool(name="p1small", bufs=4))
            psum = ph.enter_context(tc.tile_pool(name="p1psum", bufs=4,
                                                 space="PSUM"))
            for to in range(16):
                xt = work.tile([128, D], f32, tag="xt")
                nc.sync.dma_start(xt[:], x_in[to * 128:(to + 1) * 128, :])
                rstd, nmr = _ln_stats(nc, small, xt, D, epst[:])
                xn = work.tile([128, D], f32, tag="xn")
                nc.scalar.activation(xn[:], xt[:], AF.Identity,
                                     bias=nmr[:], scale=rstd[:])
                for do in range(8):
                    ptr = psum.tile([128, 128], f32, tag="ptr")
                    nc.tensor.transpose(ptr[:], xn[:, do * 128:(do + 1) * 128],
                                        ident[:])
                    nc.vector.tensor_scalar(
                        xnT[:, do, to * 128:(to + 1) * 128], ptr[:],
                        g1t[:, do:do + 1], be1t[:, do:do + 1],
                        ALU.mult, ALU.add)

        # ================= Phase 2: QKV projections =================
        with ExitStack() as ph:
            wpool = ph.enter_context(tc.tile_pool(name="p2w", bufs=2))
            psum = ph.enter_context(tc.tile_pool(name="p2psum", bufs=4,
                                                 space="PSUM"))
            wq_sb = wpool.tile([128, 8, HL * DH], f32r, tag="w")
            nc.sync.dma_start(wq_sb[:], wq_in[:].bitcast(f32r))
            wk_sb = wpool.tile([128, 8, HL * DH], f32r, tag="w")
            nc.sync.dma_start(wk_sb[:], wk_in[:].bitcast(f32r))
            for dst, w_sb in ((qT, wq_sb), (kT, wk_sb)):
                for mo in range(2):
                    for no in range(4):
                        pq = psum.tile([128, 512], f32, tag="pq")
                        for ko in range(8):
                            nc.tensor.matmul(
                                pq[:],
                                w_sb[:, ko, mo * 128:(mo + 1) * 128],
                                xnT[:, ko, no * 512:(no + 1) * 512],
                                start=(ko == 0), stop=(ko == 7))
                        nc.any.tensor_copy(out=dst[:, mo, no * 512:(no + 1) * 512],
                                           in_=pq[:])
            wv_sb = wpool.tile([128, 8, HL * DH], f32r, tag="w")
            nc.sync.dma_start(wv_sb[:], wv_in[:].bitcast(f32r))
            for to in range(16):
                pv = psum.tile([128, 256], f32, tag="pv")
                for ko in range(8):
                    nc.tensor.matmul(pv[:],
                                     xnT[:, ko, to * 128:(to + 1) * 128],
                                     wv_sb[:, ko, :],
                                     start=(ko == 0), stop=(ko == 7))
                nc.vector.tensor_copy(
                    out=v_sb[:, to, :, 0:DH],
                    in_=pv[:].rearrange("p (h d) -> p h d", h=HL))
            nc.vector.tensor_copy(out=v_sb[:, :, :, DH:DH + 1],
                                  in_=nc.const_aps.tensor(1.0, (128, 16, HL, 1), f32))

        # ================= Phase 3: causal attention =================
        with ExitStack() as ph:
            epool = ph.enter_context(tc.tile_pool(name="p3e", bufs=6))
            spool = ph.enter_context(tc.tile_pool(name="p3s", bufs=4))
            ps_s = ph.enter_context(tc.tile_pool(name="p3ps", bufs=4,
                                                 space="PSUM"))
            ps_o = ph.enter_context(tc.tile_pool(name="p3po", bufs=2,
                                                 space="PSUM"))
            for h in range(HL):
                po, ch = (h % 2) * 64, h // 2
                for jt in range(4):
                    pvo = ps_o.tile([128, 512], f32, tag="pvo")
                    n_s = 4 * jt + 4
                    for it in range(n_s):
                        pss = ps_s.tile([128, 512], f32, tag="pss")
                        nc.tensor.matmul(
                            pss[:],
                            kT[po:po + 64, ch, it * 128:(it + 1) * 128],
                            qT[po:po + 64, ch, jt * 512:(jt + 1) * 512],
                            start=True, stop=True)
                        et = epool.tile([128, 512], f32r, tag="et")
                        nc.scalar.activation(et[:], pss[:], AF.Exp, scale=0.125)
                        if it >= 4 * jt:
                            nc.vector.tensor_tensor(
                                et[:], et[:], masks[:, it - 4 * jt, :], ALU.mult)
                        nc.tensor.matmul(pvo[0:DH + 1, :],
                                         v_sb[:, it, h, :], et[:],
                                         start=(it == 0), stop=(it == n_s - 1))
                    rec = spool.tile([1, 512], f32, tag="rec")
                    nc.vector.reciprocal(rec[:], pvo[DH:DH + 1, :])
                    recb = spool.tile([64, 512], f32, tag="recb")
                    nc.gpsimd.partition_broadcast(recb[:], rec[:])
                    nc.vector.tensor_tensor(
                        oaT[po:po + 64, ch, jt * 512:(jt + 1) * 512],
                        pvo[0:DH, :], recb[:], ALU.mult)

        # ============ Phase 4: output projection partials ============
        partial_dram = dram.tile([T, D], f32)
        with ExitStack() as ph:
            work = ph.enter_context(tc.tile_pool(name="p4work", bufs=3))
            psum = ph.enter_context(tc.tile_pool(name="p4psum", bufs=4,
                                                 space="PSUM"))
            for to in range(16):
                prt = work.tile([128, D], f32, tag="prt")
                for no in range(2):
                    pp = psum.tile([128, 512], f32, tag="pp")
                    for ko in range(2):
                        nc.tensor.matmul(
                            pp[:],
                            oaT[:, ko, to * 128:(to + 1) * 128],
                            wp_sb[:, ko, no * 512:(no + 1) * 512],
                            start=(ko == 0), stop=(ko == 1))
                    nc.any.tensor_copy(out=prt[:, no * 512:(no + 1) * 512],
                                       in_=pp[:])
                nc.sync.dma_start(partial_dram[to * 128:(to + 1) * 128, :],
                                  prt[:])

        # ================= Phase 5: ReduceScatter =================
        rs_out = dram.tile([TG, D], f32)
        nc.gpsimd.collective_compute(
            "ReduceScatter", ALU.add,
            replica_groups=[[0, 1, 2, 3], [4, 5, 6, 7]],
            ins=[partial_dram[:].opt()],
            outs=[rs_out[:].opt()])

        # ======= Phase 6: residual + LN2 + transpose (512 rows) =======
        x2 = mid.tile([128, 4, D], f32, tag="mid")
        xn2T = mid.tile([128, 8, TG], f32r, tag="mid")
        with ExitStack() as ph:
            work = ph.enter_context(tc.tile_pool(name="p6work", bufs=2))
            small = ph.enter_context(tc.tile_pool(name="p6small", bufs=4))
            psum = ph.enter_context(tc.tile_pool(name="p6psum", bufs=4,
                                                 space="PSUM"))
            bp_row = work.tile([1, D], f32, tag="brow", bufs=1)
            nc.sync.dma_start(bp_row[:], bp_in[:])
            bpb = work.tile([128, D], f32, tag="bpb", bufs=1)
            nc.gpsimd.partition_broadcast(bpb[:], bp_row[:])
            b2_row = work.tile([1, D], f32, tag="brow2", bufs=1)
            nc.sync.dma_start(b2_row[:], b2_in[:])
            b2b = work.tile([128, D], f32, tag="b2b", bufs=1)
            nc.gpsimd.partition_broadcast(b2b[:], b2_row[:])
            for t2 in range(4):
                prt = work.tile([128, D], f32, tag="prt")
                nc.sync.dma_start(prt[:], rs_out[t2 * 128:(t2 + 1) * 128, :])
                xrt = work.tile([128, D], f32, tag="xrt")
                nc.sync.dma_start(xrt[:], xg_in[t2 * 128:(t2 + 1) * 128, :])
                x2s = x2[:, t2, :]
                nc.vector.tensor_tensor(x2s, prt[:], xrt[:], ALU.add)
                nc.vector.tensor_tensor(x2s, x2s, bpb[:], ALU.add)
                rstd, nmr = _ln_stats(nc, small, x2s, D, epst[:])
                xn2 = work.tile([128, D], f32, tag="xn2")
                nc.scalar.activation(xn2[:], x2s, AF.Identity,
                                     bias=nmr[:], scale=rstd[:])
                for do in range(8):
                    ptr = psum.tile([128, 128], f32, tag="ptr")
                    nc.tensor.transpose(ptr[:], xn2[:, do * 128:(do + 1) * 128],
                                        ident[:])
                    nc.vector.tensor_scalar(
                        xn2T[:, do, t2 * 128:(t2 + 1) * 128], ptr[:],
                        g2t[:, do:do + 1], be2t[:, do:do + 1],
                        ALU.mult, ALU.add)
                # fold b2 into the residual copy for the FFN epilogue
                nc.vector.tensor_tensor(x2s, x2s, b2b[:], ALU.add)

        # ================= Phase 7: FFN first matmul =================
        hT = persist.tile([128, 32, TG], f32r, tag="big")
        with ExitStack() as ph:
            wpool = ph.enter_context(tc.tile_pool(name="p7w", bufs=3))
            psum = ph.enter_context(tc.tile_pool(name="p7psum", bufs=4,
                                                 space="PSUM"))
            for mo in range(32):
                w1t = wpool.tile([128, 8, 128], f32r, tag="w1t")
                nc.sync.dma_start(w1t[:], w1_in[:, mo].bitcast(f32r))
                ph_ = psum.tile([128, 512], f32, tag="ph")
                for ko in range(8):
                    nc.tensor.matmul(ph_[:], w1t[:, ko, :], xn2T[:, ko, :],
                                     start=(ko == 0), stop=(ko == 7))
                nc.scalar.activation(hT[:, mo, :], ph_[:], AF.Relu,
                                     bias=b1s[:, mo:mo + 1])

        # ============ Phase 8: FFN second matmul + epilogue ============
        with ExitStack() as ph:
            wpool = ph.enter_context(tc.tile_pool(name="p8w", bufs=4))
            work = ph.enter_context(tc.tile_pool(name="p8work", bufs=2))
            psum = ph.enter_context(tc.tile_pool(name="p8psum", bufs=8,
                                                 space="PSUM"))
            py = [psum.tile([128, 512], f32, tag="py", name=f"py{i}")
                  for i in range(8)]
            for ko in range(32):
                w2t = wpool.tile([128, D], f32r, tag="w2t")
                nc.sync.dma_start(w2t[:], w2_in[ko * 128:(ko + 1) * 128, :]
                                  .bitcast(f32r))
                for m2 in range(4):
                    for no in range(2):
                        nc.tensor.matmul(
                            py[m2 * 2 + no][:],
                            hT[:, ko, m2 * 128:(m2 + 1) * 128],
                            w2t[:, no * 512:(no + 1) * 512],
                            start=(ko == 0), stop=(ko == 31))
            for m2 in range(4):
                osb = work.tile([128, D], f32, tag="osb")
                for no in range(2):
                    nc.vector.tensor_tensor(
                        osb[:, no * 512:(no + 1) * 512],
                        py[m2 * 2 + no][:], x2[:, m2, no * 512:(no + 1) * 512],
                        ALU.add)
                nc.sync.dma_start(out_dram[m2 * 128:(m2 + 1) * 128, :], osb[:])

    nc.compile()
    return nc


def _prep(inputs):
    x = np.asarray(inputs["x"], np.float32)
    wq = np.asarray(inputs["wq"], np.float32)
    wk = np.asarray(inputs["wk"], np.float32)
    wv = np.asarray(inputs["wv"], np.float32)
    wp = np.asarray(inputs["w_proj"], np.float32)
    bp = np.asarray(inputs["b_proj"], np.float32)
    w1 = np.asarray(inputs["w1"], np.float32)
    b1 = np.asarray(inputs["b1"], np.float32)
    w2 = np.asarray(inputs["w2"], np.float32)
    b2 = np.asarray(inputs["b2"], np.float32)
    ln1_g = np.asarray(inputs["ln1_g"], np.float32)
    ln1_b = np.asarray(inputs["ln1_b"], np.float32)
    ln2_g = np.asarray(inputs["ln2_g"], np.float32)
    ln2_b = np.asarray(inputs["ln2_b"], np.float32)

    w1r = np.ascontiguousarray(
        w1.reshape(8, 128, 32, 128).transpose(1, 2, 0, 3))
    b1r = np.ascontiguousarray(b1.reshape(32, 128).T)
    ln_tiles = {
        "ln1g": np.ascontiguousarray(ln1_g.reshape(8, 128).T),
        "ln1b": np.ascontiguousarray(ln1_b.reshape(8, 128).T),
        "ln2g": np.ascontiguousarray(ln2_g.reshape(8, 128).T),
        "ln2b": np.ascontiguousarray(ln2_b.reshape(8, 128).T),
    }
    in_maps = []
    for c in range(N_CORES):
        b, g = divmod(c, 4)
        h0 = HL * g
        wqc = np.concatenate([wq[h] for h in range(h0, h0 + HL)], axis=1)
        wkc = np.concatenate([wk[h] for h in range(h0, h0 + HL)], axis=1)
        wvc = np.concatenate([wv[h] for h in range(h0, h0 + HL)], axis=1)
        wpc = wp[h0 * DH:(h0 + HL) * DH, :]
        in_maps.append({
            "x": np.ascontiguousarray(x[b]),
            "xg": np.ascontiguousarray(x[b, g * TG:(g + 1) * TG, :]),
            "wq": np.ascontiguousarray(
                wqc.reshape(8, 128, HL * DH).transpose(1, 0, 2)),
            "wk": np.ascontiguousarray(
                wkc.reshape(8, 128, HL * DH).transpose(1, 0, 2)),
            "wv": np.ascontiguousarray(
                wvc.reshape(8, 128, HL * DH).transpose(1, 0, 2)),
            "wp": np.ascontiguousarray(
                wpc.reshape(2, 128, D).transpose(1, 0, 2)),
            "bp": np.ascontiguousarray(bp.reshape(1, D)),
            "w1": w1r,
            "b1": b1r,
            "w2": w2,
            "b2": np.ascontiguousarray(b2.reshape(1, D)),
            **ln_tiles,
        })
    return in_maps


def _make_runner(nc):
    """Build a cached jitted SPMD executor (mirrors bass2jax.run_bass_via_pjrt
    but jits once and is reused across kernel() calls)."""
    import jax
    from jax.experimental.shard_map import shard_map
    from jax.sharding import Mesh, PartitionSpec
    from concourse import bass2jax as b2j

    b2j.install_neuronx_cc_hook()
    partition_name = (nc.partition_id_tensor.name
                      if nc.partition_id_tensor else None)
    in_names, out_names, out_avals, zero_shapes = [], [], [], []
    for alloc in nc.m.functions[0].allocations:
        if not isinstance(alloc, mybir.MemoryLocationSet):
            continue
        name = alloc.memorylocations[0].name
        if alloc.kind == "ExternalInput":
            if name != partition_name:
                in_names.append(name)
        elif alloc.kind == "ExternalOutput":
            shape = tuple(alloc.tensor_shape)
            dtype = mybir.dt.np(alloc.dtype)
            out_names.append(name)
            out_avals.append(jax.core.ShapedArray(shape, dtype))
            zero_shapes.append((shape, dtype))
    n_params = len(in_names)
    n_outs = len(out_avals)
    all_in_names = list(in_names) + list(out_names)
    if partition_name is not None:
        all_in_names.append(partition_name)
    donate = tuple(range(n_params, n_params + n_outs))

    def _body(*args):
        operands = list(args)
        if partition_name is not None:
            operands.append(b2j.partition_id_tensor())
        outs = b2j._bass_exec_p.bind(
            *operands,
            out_avals=tuple(out_avals),
            in_names=tuple(all_in_names),
            out_names=tuple(out_names),
            lowering_input_output_aliases=(),
            sim_require_finite=True,
            sim_require_nnan=True,
            nc=nc,
        )
        return tuple(outs)

    devices = jax.devices()[:N_CORES]
    mesh = Mesh(np.asarray(devices), ("core",))
    in_specs = (PartitionSpec("core"),) * (n_params + n_outs)
    out_specs = (PartitionSpec("core"),) * n_outs
    sharded = jax.jit(
        shard_map(_body, mesh=mesh, in_specs=in_specs, out_specs=out_specs,
                  check_rep=False),
        donate_argnums=donate, keep_unused=True)

    def run(in_maps):
        concat_in = [
            np.concatenate([np.asarray(in_maps[c][name])
                            for c in range(N_CORES)], axis=0)
            for name in in_names
        ]
        concat_zeros = [
            np.zeros((N_CORES * s[0], *s[1:]), dt) for s, dt in zero_shapes
        ]
        out_arrs = sharded(*concat_in, *concat_zeros)
        return [
            {name: np.asarray(out_arrs[i]).reshape(N_CORES,
                                                   *zero_shapes[i][0])[c]
             for i, name in enumerate(out_names)}
            for c in range(N_CORES)
        ]

    return run


_CACHE = {}


def _get_nc():
    if "nc" not in _CACHE:
        _CACHE["nc"] = _build()
    return _CACHE["nc"]


def _get_runner():
    if "run" not in _CACHE:
        _CACHE["run"] = _make_runner(_get_nc())
    return _CACHE["run"]


def kernel(**inputs):
    run = _get_runner()
    in_maps = _prep(inputs)
    res = run(in_maps)
    B = 2
    out = np.empty((B, T, D), np.float32)
    for c in range(N_CORES):
        b, g = divmod(c, 4)
        out[b, g * TG:(g + 1) * TG, :] = res[c]["out"]
    return out


# revision 11
# speedup vs baseline: 1.0562x; 1.0562x over previous
"""Distributed Trainium2 (Bass/Tile) kernel for a pre-LN transformer block.

Reference computation (per batch element):
    xn = LN1(x); q,k,v = per-head projections of xn
    attn = causal-softmax(q k^T / sqrt(dh)) v
    x1 = x + concat_heads(attn) @ w_proj + b_proj
    out = x1 + relu(LN2(x1) @ w1 + b1) @ w2 + b2

Sharding over 8 NeuronCores: core c handles batch b=c//4 and head group
g=c%4 (4 of 16 heads).  Attention + projection partials are head-parallel;
chunked ReduceScatter(add) collectives over each 4-core group turn the
projection partials into per-core 512-row slices of x1 (pipelined with the
projection matmuls); the FFN then runs sequence-parallel (512 rows per
core) with no further communication.  The host assembles the 8 [512,1024]
outputs into the full [2,2048,1024] result.

Matmul operands are bf16 (fast weight loads, fp32 PSUM accumulation);
layernorm statistics, softmax denominators, residuals and the collective
run in fp32.
"""

import numpy as np
import ml_dtypes

import concourse.bass as bass
import concourse.mybir as mybir
import concourse.tile as tile
from contextlib import ExitStack
from concourse import bacc
from concourse.masks import make_identity
from concourse.bass_utils import run_bass_kernel_spmd

T = 2048          # sequence length
D = 1024          # embedding dim
H = 16            # total heads
DH = 64           # head dim
HL = 4            # heads per core
TG = 512          # rows per core in the FFN phase
DF = 4096         # FFN hidden dim
EPS = 1e-5
N_CORES = 8

f32 = mybir.dt.float32
bf16 = mybir.dt.bfloat16
AF = mybir.ActivationFunctionType
ALU = mybir.AluOpType
BF16 = ml_dtypes.bfloat16


def _ln_stats(nc, pool, xt, width, eps_ap):
    """Per-partition mean/var over `width` free elements -> (rstd, neg_mu_rstd)."""
    nchunk = width // 512
    bns = pool.tile([128, nchunk, 6], f32, tag="bns")
    for i in range(nchunk):
        nc.vector.bn_stats(bns[:, i, :], xt[:, i * 512:(i + 1) * 512])
    agg = pool.tile([128, 2], f32, tag="agg")
    nc.vector.bn_aggr(agg[:], bns[:].rearrange("p a b -> p (a b)"))
    std = pool.tile([128, 1], f32, tag="std")
    nc.scalar.activation(std[:], agg[:, 1:2], AF.Sqrt, bias=eps_ap)
    rstd = pool.tile([128, 1], f32, tag="rstd")
    nc.vector.reciprocal(rstd[:], std[:])
    nmr = pool.tile([128, 1], f32, tag="nmr")
    nc.vector.tensor_scalar(nmr[:], agg[:, 0:1], rstd[:], -1.0, ALU.mult, ALU.mult)
    return rstd, nmr


def _build():
    nc = bacc.Bacc("TRN2", target_bir_lowering=False, debug=False,
                   num_devices=N_CORES)

    x_in = nc.dram_tensor("x", [T, D], f32, kind="ExternalInput")
    wq_in = nc.dram_tensor("wq", [128, 8, HL * DH], bf16, kind="ExternalInput")
    wk_in = nc.dram_tensor("wk", [128, 8, HL * DH], bf16, kind="ExternalInput")
    wv_in = nc.dram_tensor("wv", [128, 8, HL * DH], bf16, kind="ExternalInput")
    wp_in = nc.dram_tensor("wp", [128, 2, D], bf16, kind="ExternalInput")
    bp_in = nc.dram_tensor("bp", [1, D], f32, kind="ExternalInput")
    w1_in = nc.dram_tensor("w1", [128, 32, 8, 128], bf16, kind="ExternalInput")
    b1_in = nc.dram_tensor("b1", [128, 32], f32, kind="ExternalInput")
    w2_in = nc.dram_tensor("w2", [DF, D], bf16, kind="ExternalInput")
    b2_in = nc.dram_tensor("b2", [1, D], f32, kind="ExternalInput")
    g1_in = nc.dram_tensor("ln1g", [128, 8], f32, kind="ExternalInput")
    be1_in = nc.dram_tensor("ln1b", [128, 8], f32, kind="ExternalInput")
    g2_in = nc.dram_tensor("ln2g", [128, 8], f32, kind="ExternalInput")
    be2_in = nc.dram_tensor("ln2b", [128, 8], f32, kind="ExternalInput")
    xg_in = nc.dram_tensor("xg", [TG, D], f32, kind="ExternalInput")
    out_dram = nc.dram_tensor("out", [TG, D], f32, kind="ExternalOutput")

    with tile.TileContext(nc) as tc, ExitStack() as top:
        persist = top.enter_context(tc.tile_pool(name="persist", bufs=1))
        mid = top.enter_context(tc.tile_pool(name="mid", bufs=4))
        consts = top.enter_context(tc.tile_pool(name="consts", bufs=1))
        dram = top.enter_context(tc.tile_pool(name="dram", bufs=1, space="DRAM"))

        # ---- constants ----
        ident = consts.tile([128, 128], bf16, tag="ident")
        make_identity(nc, ident[:])
        masks = consts.tile([128, HL, 512], bf16, tag="masks")
        nc.vector.memset(masks[:], 1.0)
        for d in range(HL):
            # keep (1.0) where global_t - global_s >= 0, i.e. f - p - 128*d >= 0
            nc.gpsimd.affine_select(
                out=masks[:, d, :], in_=masks[:, d, :],
                compare_op=ALU.is_ge, fill=0.0,
                base=-128 * d, pattern=[[1, 512]], channel_multiplier=-1)
        g1t = consts.tile([128, 8], f32, tag="g1t")
        nc.sync.dma_start(g1t[:], g1_in[:])
        be1t = consts.tile([128, 8], f32, tag="be1t")
        nc.sync.dma_start(be1t[:], be1_in[:])
        g2t = consts.tile([128, 8], f32, tag="g2t")
        nc.sync.dma_start(g2t[:], g2_in[:])
        be2t = consts.tile([128, 8], f32, tag="be2t")
        nc.sync.dma_start(be2t[:], be2_in[:])
        epst = consts.tile([128, 1], f32, tag="epst")
        nc.vector.memset(epst[:], EPS)
        b1s = consts.tile([128, 32], f32, tag="b1s")
        nc.sync.dma_start(b1s[:], b1_in[:])
        wp_sb = consts.tile([128, 2, D], bf16, tag="wp_sb")
        nc.sync.dma_start(wp_sb[:], wp_in[:])

        # ---- persistent activation tiles ----
        xnT = persist.tile([128, 8, T], bf16, tag="big")        # LN1(x)^T
        qT = mid.tile([128, 2, T], bf16, tag="mid")             # q^T (4 heads)
        kT = mid.tile([128, 2, T], bf16, tag="mid")             # k^T
        v_sb = mid.tile([128, 16, HL, DH + 1], bf16, tag="mid")  # v + ones col
        oaT = mid.tile([128, 2, T], bf16, tag="mid")            # attn out^T

        # ================= Phase 1: LN1 + transpose =================
        with ExitStack() as ph:
            work = ph.enter_context(tc.tile_pool(name="p1work", bufs=3))
            small = ph.enter_context(tc.tile_pool(name="p1small", bufs=4))
            psum = ph.enter_context(tc.tile_pool(name="p1psum", bufs=6,
                                                 space="PSUM"))
            for to in range(16):
                xt = work.tile([128, D], f32, tag="xt")
                nc.sync.dma_start(xt[:], x_in[to * 128:(to + 1) * 128, :])
                rstd, nmr = _ln_stats(nc, small, xt, D, epst[:])
                xn = work.tile([128, D], bf16, tag="xn")
                nc.scalar.activation(xn[:], xt[:], AF.Identity,
                                     bias=nmr[:], scale=rstd[:])
                for do in range(8):
                    ptr = psum.tile([128, 128], bf16, tag="ptr")
                    nc.tensor.transpose(ptr[:], xn[:, do * 128:(do + 1) * 128],
                                        ident[:])
                    nc.vector.tensor_scalar(
                        xnT[:, do, to * 128:(to + 1) * 128], ptr[:],
                        g1t[:, do:do + 1], be1t[:, do:do + 1],
                        ALU.mult, ALU.add)

        # ================= Phase 2: QKV projections =================
        with ExitStack() as ph:
            wpool = ph.enter_context(tc.tile_pool(name="p2w", bufs=2))
            psum = ph.enter_context(tc.tile_pool(name="p2psum", bufs=4,
                                                 space="PSUM"))
            wq_sb = wpool.tile([128, 8, HL * DH], bf16, tag="w")
            nc.sync.dma_start(wq_sb[:], wq_in[:])
            wk_sb = wpool.tile([128, 8, HL * DH], bf16, tag="w")
            nc.sync.dma_start(wk_sb[:], wk_in[:])
            for dst, w_sb in ((qT, wq_sb), (kT, wk_sb)):
                for mo in range(2):
                    for no in range(4):
                        pq = psum.tile([128, 512], f32, tag="pq")
                        for ko in range(8):
                            nc.tensor.matmul(
                                pq[:],
                                w_sb[:, ko, mo * 128:(mo + 1) * 128],
                                xnT[:, ko, no * 512:(no + 1) * 512],
                                start=(ko == 0), stop=(ko == 7))
                        nc.vector.tensor_copy(
                            out=dst[:, mo, no * 512:(no + 1) * 512], in_=pq[:])
            wv_sb = wpool.tile([128, 8, HL * DH], bf16, tag="w")
            nc.sync.dma_start(wv_sb[:], wv_in[:])
            for to in range(16):
                pv = psum.tile([128, 256], f32, tag="pv")
                for ko in range(8):
                    nc.tensor.matmul(pv[:],
                                     xnT[:, ko, to * 128:(to + 1) * 128],
                                     wv_sb[:, ko, :],
                                     start=(ko == 0), stop=(ko == 7))
                nc.vector.tensor_copy(
                    out=v_sb[:, to, :, 0:DH],
                    in_=pv[:].rearrange("p (h d) -> p h d", h=HL))
            nc.vector.tensor_copy(out=v_sb[:, :, :, DH:DH + 1],
                                  in_=nc.const_aps.tensor(1.0, (128, 16, HL, 1), f32))

        # ================= Phase 3: causal attention =================
        with ExitStack() as ph:
            epool = ph.enter_context(tc.tile_pool(name="p3e", bufs=8))
            spool = ph.enter_context(tc.tile_pool(name="p3s", bufs=4))
            ps_s = ph.enter_context(tc.tile_pool(name="p3ps", bufs=4,
                                                 space="PSUM"))
            ps_o = ph.enter_context(tc.tile_pool(name="p3po", bufs=2,
                                                 space="PSUM"))
            for h in range(HL):
                po, ch = (h % 2) * 64, h // 2
                for jt in range(4):
                    pvo = ps_o.tile([128, 512], f32, tag="pvo")
                    n_s = 4 * jt + 4
                    for it in range(n_s):
                        pss = ps_s.tile([128, 512], f32, tag="pss")
                        nc.tensor.matmul(
                            pss[:],
                            kT[po:po + 64, ch, it * 128:(it + 1) * 128],
                            qT[po:po + 64, ch, jt * 512:(jt + 1) * 512],
                            start=True, stop=True)
                        et = epool.tile([128, 512], bf16, tag="et")
                        nc.scalar.activation(et[:], pss[:], AF.Exp, scale=0.125)
                        if it >= 4 * jt:
                            nc.vector.tensor_tensor(
                                et[:], et[:], masks[:, it - 4 * jt, :], ALU.mult)
                        nc.tensor.matmul(pvo[0:DH + 1, :],
                                         v_sb[:, it, h, :], et[:],
                                         start=(it == 0), stop=(it == n_s - 1))
                    lrow = spool.tile([1, 512], f32, tag="lrow")
                    nc.scalar.copy(lrow[:], pvo[DH:DH + 1, :])
                    lb = spool.tile([64, 512], f32, tag="lb")
                    nc.gpsimd.partition_broadcast(lb[:], lrow[:])
                    nc.vector.reciprocal(lb[:], lb[:])
                    nc.vector.tensor_tensor(
                        oaT[po:po + 64, ch, jt * 512:(jt + 1) * 512],
                        pvo[0:DH, :], lb[:], ALU.mult)

        # ==== Phase 4: output projection partials + chunked RS ====
        # Chunk k holds, for each group rank r, rows [512r+128k, 512r+128k+128)
        # of the projection partial, so ReduceScatter hands rank r its rows.
        partial_perm = [dram.tile([TG, D], f32, name=f"pperm{k}")
                        for k in range(4)]
        rs_chunks = [dram.tile([128, D], f32, name=f"rsc{k}") for k in range(4)]
        with ExitStack() as ph:
            work = ph.enter_context(tc.tile_pool(name="p4work", bufs=3))
            psum = ph.enter_context(tc.tile_pool(name="p4psum", bufs=4,
                                                 space="PSUM"))
            for k in range(4):
                for r in range(4):
                    to = 4 * r + k
                    prt = work.tile([128, D], f32, tag="prt")
                    for no in range(2):
                        pp = psum.tile([128, 512], f32, tag="pp")
                        for ko in range(2):
                            nc.tensor.matmul(
                                pp[:],
                                oaT[:, ko, to * 128:(to + 1) * 128],
                                wp_sb[:, ko, no * 512:(no + 1) * 512],
                                start=(ko == 0), stop=(ko == 1))
                        nc.vector.tensor_copy(
                            out=prt[:, no * 512:(no + 1) * 512], in_=pp[:])
                    nc.sync.dma_start(
                        partial_perm[k][r * 128:(r + 1) * 128, :], prt[:])
                nc.gpsimd.collective_compute(
                    "ReduceScatter", ALU.add,
                    replica_groups=[[0, 1, 2, 3], [4, 5, 6, 7]],
                    ins=[partial_perm[k][:].opt()],
                    outs=[rs_chunks[k][:].opt()])

        # ======= Phase 6: residual + LN2 + transpose (512 rows) =======
        x2 = mid.tile([128, 4, D], f32, tag="mid")
        xn2T = mid.tile([128, 8, TG], bf16, tag="mid")
        with ExitStack() as ph:
            work = ph.enter_context(tc.tile_pool(name="p6work", bufs=2))
            small = ph.enter_context(tc.tile_pool(name="p6small", bufs=4))
            psum = ph.enter_context(tc.tile_pool(name="p6psum", bufs=6,
                                                 space="PSUM"))
            bp_row = work.tile([1, D], f32, tag="brow", bufs=1)
            nc.sync.dma_start(bp_row[:], bp_in[:])
            bpb = work.tile([128, D], f32, tag="bpb", bufs=1)
            nc.gpsimd.partition_broadcast(bpb[:], bp_row[:])
            b2_row = work.tile([1, D], f32, tag="brow2", bufs=1)
            nc.sync.dma_start(b2_row[:], b2_in[:])
            b2b = work.tile([128, D], f32, tag="b2b", bufs=1)
            nc.gpsimd.partition_broadcast(b2b[:], b2_row[:])
            for t2 in range(4):
                prt = work.tile([128, D], f32, tag="prt")
                nc.sync.dma_start(prt[:], rs_chunks[t2][:])
                xrt = work.tile([128, D], f32, tag="xrt")
                nc.sync.dma_start(xrt[:], xg_in[t2 * 128:(t2 + 1) * 128, :])
                x2s = x2[:, t2, :]
                nc.vector.tensor_tensor(x2s, prt[:], xrt[:], ALU.add)
                nc.vector.tensor_tensor(x2s, x2s, bpb[:], ALU.add)
                rstd, nmr = _ln_stats(nc, small, x2s, D, epst[:])
                xn2 = work.tile([128, D], bf16, tag="xn2")
                nc.scalar.activation(xn2[:], x2s, AF.Identity,
                                     bias=nmr[:], scale=rstd[:])
                for do in range(8):
                    ptr = psum.tile([128, 128], bf16, tag="ptr")
                    nc.tensor.transpose(ptr[:], xn2[:, do * 128:(do + 1) * 128],
                                        ident[:])
                    nc.vector.tensor_scalar(
                        xn2T[:, do, t2 * 128:(t2 + 1) * 128], ptr[:],
                        g2t[:, do:do + 1], be2t[:, do:do + 1],
                        ALU.mult, ALU.add)
                # fold b2 into the residual copy for the FFN epilogue
                nc.vector.tensor_tensor(x2s, x2s, b2b[:], ALU.add)

        # ================= Phase 7: FFN first matmul =================
        hT = persist.tile([128, 32, TG], bf16, tag="big")
        with ExitStack() as ph:
            wpool = ph.enter_context(tc.tile_pool(name="p7w", bufs=4))
            psum = ph.enter_context(tc.tile_pool(name="p7psum", bufs=6,
                                                 space="PSUM"))
            for mo in range(32):
                w1t = wpool.tile([128, 8, 128], bf16, tag="w1t")
                nc.sync.dma_start(w1t[:], w1_in[:, mo])
                ph_ = psum.tile([128, 512], f32, tag="ph")
                for ko in range(8):
                    nc.tensor.matmul(ph_[:], w1t[:, ko, :], xn2T[:, ko, :],
                                     start=(ko == 0), stop=(ko == 7))
                nc.scalar.activation(hT[:, mo, :], ph_[:], AF.Relu,
                                     bias=b1s[:, mo:mo + 1])

        # ============ Phase 8: FFN second matmul + epilogue ============
        with ExitStack() as ph:
            wpool = ph.enter_context(tc.tile_pool(name="p8w", bufs=6))
            work = ph.enter_context(tc.tile_pool(name="p8work", bufs=2))
            psum = ph.enter_context(tc.tile_pool(name="p8psum", bufs=8,
                                                 space="PSUM"))
            py = [psum.tile([128, 512], f32, tag="py", name=f"py{i}")
                  for i in range(8)]
            for ko in range(32):
                w2t = wpool.tile([128, D], bf16, tag="w2t")
                nc.sync.dma_start(w2t[:], w2_in[ko * 128:(ko + 1) * 128, :])
                for m2 in range(4):
                    for no in range(2):
                        nc.tensor.matmul(
                            py[m2 * 2 + no][:],
                            hT[:, ko, m2 * 128:(m2 + 1) * 128],
                            w2t[:, no * 512:(no + 1) * 512],
                            start=(ko == 0), stop=(ko == 31))
            for m2 in range(4):
                osb = work.tile([128, D], f32, tag="osb")
                for no in range(2):
                    nc.vector.tensor_tensor(
                        osb[:, no * 512:(no + 1) * 512],
                        py[m2 * 2 + no][:], x2[:, m2, no * 512:(no + 1) * 512],
                        ALU.add)
                nc.sync.dma_start(out_dram[m2 * 128:(m2 + 1) * 128, :], osb[:])

    nc.compile()
    return nc


def _prep(inputs):
    x = np.asarray(inputs["x"], np.float32)
    wq = np.asarray(inputs["wq"], np.float32)
    wk = np.asarray(inputs["wk"], np.float32)
    wv = np.asarray(inputs["wv"], np.float32)
    wp = np.asarray(inputs["w_proj"], np.float32)
    bp = np.asarray(inputs["b_proj"], np.float32)
    w1 = np.asarray(inputs["w1"], np.float32)
    b1 = np.asarray(inputs["b1"], np.float32)
    w2 = np.asarray(inputs["w2"], np.float32)
    b2 = np.asarray(inputs["b2"], np.float32)
    ln1_g = np.asarray(inputs["ln1_g"], np.float32)
    ln1_b = np.asarray(inputs["ln1_b"], np.float32)
    ln2_g = np.asarray(inputs["ln2_g"], np.float32)
    ln2_b = np.asarray(inputs["ln2_b"], np.float32)

    w1r = np.ascontiguousarray(
        w1.reshape(8, 128, 32, 128).transpose(1, 2, 0, 3)).astype(BF16)
    w2r = w2.astype(BF16)
    b1r = np.ascontiguousarray(b1.reshape(32, 128).T)
    ln_tiles = {
        "ln1g": np.ascontiguousarray(ln1_g.reshape(8, 128).T),
        "ln1b": np.ascontiguousarray(ln1_b.reshape(8, 128).T),
        "ln2g": np.ascontiguousarray(ln2_g.reshape(8, 128).T),
        "ln2b": np.ascontiguousarray(ln2_b.reshape(8, 128).T),
    }
    in_maps = []
    for c in range(N_CORES):
        b, g = divmod(c, 4)
        h0 = HL * g
        wqc = np.concatenate([wq[h] for h in range(h0, h0 + HL)], axis=1)
        wkc = np.concatenate([wk[h] for h in range(h0, h0 + HL)], axis=1)
        wvc = np.concatenate([wv[h] for h in range(h0, h0 + HL)], axis=1)
        wpc = wp[h0 * DH:(h0 + HL) * DH, :]
        in_maps.append({
            "x": np.ascontiguousarray(x[b]),
            "xg": np.ascontiguousarray(x[b, g * TG:(g + 1) * TG, :]),
            "wq": np.ascontiguousarray(
                wqc.reshape(8, 128, HL * DH).transpose(1, 0, 2)).astype(BF16),
            "wk": np.ascontiguousarray(
                wkc.reshape(8, 128, HL * DH).transpose(1, 0, 2)).astype(BF16),
            "wv": np.ascontiguousarray(
                wvc.reshape(8, 128, HL * DH).transpose(1, 0, 2)).astype(BF16),
            "wp": np.ascontiguousarray(
                wpc.reshape(2, 128, D).transpose(1, 0, 2)).astype(BF16),
            "bp": np.ascontiguousarray(bp.reshape(1, D)),
            "w1": w1r,
            "b1": b1r,
            "w2": w2r,
            "b2": np.ascontiguousarray(b2.reshape(1, D)),
            **ln_tiles,
        })
    return in_maps


def _make_runner(nc):
    """Build a cached jitted SPMD executor (mirrors bass2jax.run_bass_via_pjrt
    but jits once and is reused across kernel() calls)."""
    import jax
    from jax.experimental.shard_map import shard_map
    from jax.sharding import Mesh, PartitionSpec
    from concourse import bass2jax as b2j

    b2j.install_neuronx_cc_hook()
    partition_name = (nc.partition_id_tensor.name
                      if nc.partition_id_tensor else None)
    in_names, out_names, out_avals, zero_shapes = [], [], [], []
    for alloc in nc.m.functions[0].allocations:
        if not isinstance(alloc, mybir.MemoryLocationSet):
            continue
        name = alloc.memorylocations[0].name
        if alloc.kind == "ExternalInput":
            if name != partition_name:
                in_names.append(name)
        elif alloc.kind == "ExternalOutput":
            shape = tuple(alloc.tensor_shape)
            dtype = mybir.dt.np(alloc.dtype)
            out_names.append(name)
            out_avals.append(jax.core.ShapedArray(shape, dtype))
            zero_shapes.append((shape, dtype))
    n_params = len(in_names)
    n_outs = len(out_avals)
    all_in_names = list(in_names) + list(out_names)
    if partition_name is not None:
        all_in_names.append(partition_name)
    donate = tuple(range(n_params, n_params + n_outs))

    def _body(*args):
        operands = list(args)
        if partition_name is not None:
            operands.append(b2j.partition_id_tensor())
        outs = b2j._bass_exec_p.bind(
            *operands,
            out_avals=tuple(out_avals),
            in_names=tuple(all_in_names),
            out_names=tuple(out_names),
            lowering_input_output_aliases=(),
            sim_require_finite=True,
            sim_require_nnan=True,
            nc=nc,
        )
        return tuple(outs)

    devices = jax.devices()[:N_CORES]
    mesh = Mesh(np.asarray(devices), ("core",))
    in_specs = (PartitionSpec("core"),) * (n_params + n_outs)
    out_specs = (PartitionSpec("core"),) * n_outs
    sharded = jax.jit(
        shard_map(_body, mesh=mesh, in_specs=in_specs, out_specs=out_specs,
                  check_rep=False),
        donate_argnums=donate, keep_unused=True)

    def run(in_maps):
        concat_in = [
            np.concatenate([np.asarray(in_maps[c][name])
                            for c in range(N_CORES)], axis=0)
            for name in in_names
        ]
        concat_zeros = [
            np.zeros((N_CORES * s[0], *s[1:]), dt) for s, dt in zero_shapes
        ]
        out_arrs = sharded(*concat_in, *concat_zeros)
        return [
            {name: np.asarray(out_arrs[i]).reshape(N_CORES,
                                                   *zero_shapes[i][0])[c]
             for i, name in enumerate(out_names)}
            for c in range(N_CORES)
        ]

    return run


_CACHE = {}


def _get_nc():
    if "nc" not in _CACHE:
        _CACHE["nc"] = _build()
    return _CACHE["nc"]


def _get_runner():
    if "run" not in _CACHE:
        _CACHE["run"] = _make_runner(_get_nc())
    return _CACHE["run"]


def kernel(**inputs):
    run = _get_runner()
    in_maps = _prep(inputs)
    res = run(in_maps)
    B = 2
    out = np.empty((B, T, D), np.float32)
    for c in range(N_CORES):
        b, g = divmod(c, 4)
        out[b, g * TG:(g + 1) * TG, :] = res[c]["out"]
    return out


# revision 12
# speedup vs baseline: 1.1421x; 1.0813x over previous
"""Distributed Trainium2 (Bass/Tile) kernel for a pre-LN transformer block.

Reference computation (per batch element):
    xn = LN1(x); q,k,v = per-head projections of xn
    attn = causal-softmax(q k^T / sqrt(dh)) v
    x1 = x + concat_heads(attn) @ w_proj + b_proj
    out = x1 + relu(LN2(x1) @ w1 + b1) @ w2 + b2

Sharding over 8 NeuronCores: core c handles batch b=c//4 and head group
g=c%4 (4 of 16 heads).  Attention + projection partials are head-parallel;
chunked ReduceScatter(add) collectives over each 4-core group turn the
projection partials into per-core 512-row slices of x1 (pipelined with the
projection matmuls); the FFN then runs sequence-parallel (512 rows per
core) with no further communication.  The host assembles the 8 [512,1024]
outputs into the full [2,2048,1024] result.

Matmul operands are bf16 (fast weight loads, fp32 PSUM accumulation);
layernorm statistics, softmax denominators, residuals and the collective
run in fp32.
"""

import numpy as np
import ml_dtypes

import concourse.bass as bass
import concourse.mybir as mybir
import concourse.tile as tile
from contextlib import ExitStack
from concourse import bacc
from concourse.masks import make_identity
from concourse.bass_utils import run_bass_kernel_spmd

T = 2048          # sequence length
D = 1024          # embedding dim
H = 16            # total heads
DH = 64           # head dim
HL = 4            # heads per core
TG = 512          # rows per core in the FFN phase
DF = 4096         # FFN hidden dim
EPS = 1e-5
N_CORES = 8

f32 = mybir.dt.float32
bf16 = mybir.dt.bfloat16
AF = mybir.ActivationFunctionType
ALU = mybir.AluOpType
BF16 = ml_dtypes.bfloat16


def _ln_stats(nc, pool, xt, width, eps_ap):
    """Per-partition mean/var over `width` free elements -> (rstd, neg_mu_rstd)."""
    nchunk = width // 512
    bns = pool.tile([128, nchunk, 6], f32, tag="bns")
    for i in range(nchunk):
        nc.vector.bn_stats(bns[:, i, :], xt[:, i * 512:(i + 1) * 512])
    agg = pool.tile([128, 2], f32, tag="agg")
    nc.vector.bn_aggr(agg[:], bns[:].rearrange("p a b -> p (a b)"))
    std = pool.tile([128, 1], f32, tag="std")
    nc.scalar.activation(std[:], agg[:, 1:2], AF.Sqrt, bias=eps_ap)
    rstd = pool.tile([128, 1], f32, tag="rstd")
    nc.vector.reciprocal(rstd[:], std[:])
    nmr = pool.tile([128, 1], f32, tag="nmr")
    nc.vector.tensor_scalar(nmr[:], agg[:, 0:1], rstd[:], -1.0, ALU.mult, ALU.mult)
    return rstd, nmr


def _build():
    nc = bacc.Bacc("TRN2", target_bir_lowering=False, debug=False,
                   num_devices=N_CORES)

    x_in = nc.dram_tensor("x", [T, D], f32, kind="ExternalInput")
    wq_in = nc.dram_tensor("wq", [128, 8, HL * DH], bf16, kind="ExternalInput")
    wk_in = nc.dram_tensor("wk", [128, 8, HL * DH], bf16, kind="ExternalInput")
    wv_in = nc.dram_tensor("wv", [128, 8, HL * DH], bf16, kind="ExternalInput")
    wp_in = nc.dram_tensor("wp", [128, 2, D], bf16, kind="ExternalInput")
    bp_in = nc.dram_tensor("bp", [1, D], f32, kind="ExternalInput")
    w1_in = nc.dram_tensor("w1", [128, 32, 8, 128], bf16, kind="ExternalInput")
    b1_in = nc.dram_tensor("b1", [128, 32], f32, kind="ExternalInput")
    w2_in = nc.dram_tensor("w2", [DF, D], bf16, kind="ExternalInput")
    b2_in = nc.dram_tensor("b2", [1, D], f32, kind="ExternalInput")
    g1_in = nc.dram_tensor("ln1g", [128, 8], f32, kind="ExternalInput")
    be1_in = nc.dram_tensor("ln1b", [128, 8], f32, kind="ExternalInput")
    g2_in = nc.dram_tensor("ln2g", [128, 8], f32, kind="ExternalInput")
    be2_in = nc.dram_tensor("ln2b", [128, 8], f32, kind="ExternalInput")
    xg_in = nc.dram_tensor("xg", [TG, D], f32, kind="ExternalInput")
    out_dram = nc.dram_tensor("out", [TG, D], f32, kind="ExternalOutput")

    with tile.TileContext(nc) as tc, ExitStack() as top:
        persist = top.enter_context(tc.tile_pool(name="persist", bufs=1))
        mid = top.enter_context(tc.tile_pool(name="mid", bufs=4))
        consts = top.enter_context(tc.tile_pool(name="consts", bufs=1))
        dram = top.enter_context(tc.tile_pool(name="dram", bufs=1, space="DRAM"))

        # ---- constants ----
        ident = consts.tile([128, 128], bf16, tag="ident")
        make_identity(nc, ident[:])
        masks = consts.tile([128, HL, 512], bf16, tag="masks")
        nc.vector.memset(masks[:], 1.0)
        for d in range(HL):
            # keep (1.0) where global_t - global_s >= 0, i.e. f - p - 128*d >= 0
            nc.gpsimd.affine_select(
                out=masks[:, d, :], in_=masks[:, d, :],
                compare_op=ALU.is_ge, fill=0.0,
                base=-128 * d, pattern=[[1, 512]], channel_multiplier=-1)
        g1t = consts.tile([128, 8], f32, tag="g1t")
        nc.sync.dma_start(g1t[:], g1_in[:])
        be1t = consts.tile([128, 8], f32, tag="be1t")
        nc.sync.dma_start(be1t[:], be1_in[:])
        g2t = consts.tile([128, 8], f32, tag="g2t")
        nc.sync.dma_start(g2t[:], g2_in[:])
        be2t = consts.tile([128, 8], f32, tag="be2t")
        nc.sync.dma_start(be2t[:], be2_in[:])
        epst = consts.tile([128, 1], f32, tag="epst")
        nc.vector.memset(epst[:], EPS)
        b1s = consts.tile([128, 32], f32, tag="b1s")
        nc.sync.dma_start(b1s[:], b1_in[:])
        wp_sb = consts.tile([128, 2, D], bf16, tag="wp_sb")
        nc.sync.dma_start(wp_sb[:], wp_in[:])

        # ---- persistent activation tiles ----
        xnT = persist.tile([128, 8, T], bf16, tag="big")        # LN1(x)^T
        qT = mid.tile([128, 2, T], bf16, tag="mid")             # q^T (4 heads)
        kT = mid.tile([128, 2, T], bf16, tag="mid")             # k^T
        v_sb = mid.tile([128, 16, HL, DH + 1], bf16, tag="mid")  # v + ones col
        oaT = mid.tile([128, 2, T], bf16, tag="mid")            # attn out^T

        # ================= Phase 1: LN1 + transpose =================
        with ExitStack() as ph:
            work = ph.enter_context(tc.tile_pool(name="p1work", bufs=3))
            small = ph.enter_context(tc.tile_pool(name="p1small", bufs=4))
            psum = ph.enter_context(tc.tile_pool(name="p1psum", bufs=6,
                                                 space="PSUM"))
            for to in range(16):
                xt = work.tile([128, D], f32, tag="xt")
                nc.sync.dma_start(xt[:], x_in[to * 128:(to + 1) * 128, :])
                rstd, nmr = _ln_stats(nc, small, xt, D, epst[:])
                xn = work.tile([128, D], bf16, tag="xn")
                nc.scalar.activation(xn[:], xt[:], AF.Identity,
                                     bias=nmr[:], scale=rstd[:])
                for do in range(8):
                    ptr = psum.tile([128, 128], bf16, tag="ptr")
                    nc.tensor.transpose(ptr[:], xn[:, do * 128:(do + 1) * 128],
                                        ident[:])
                    nc.scalar.activation(
                        xnT[:, do, to * 128:(to + 1) * 128], ptr[:],
                        AF.Identity, bias=be1t[:, do:do + 1],
                        scale=g1t[:, do:do + 1])

        # ================= Phase 2: QKV projections =================
        with ExitStack() as ph:
            wpool = ph.enter_context(tc.tile_pool(name="p2w", bufs=2))
            psum = ph.enter_context(tc.tile_pool(name="p2psum", bufs=4,
                                                 space="PSUM"))
            wq_sb = wpool.tile([128, 8, HL * DH], bf16, tag="w")
            nc.sync.dma_start(wq_sb[:], wq_in[:])
            wk_sb = wpool.tile([128, 8, HL * DH], bf16, tag="w")
            nc.sync.dma_start(wk_sb[:], wk_in[:])
            for dst, w_sb in ((qT, wq_sb), (kT, wk_sb)):
                for mo in range(2):
                    for no in range(4):
                        pq = psum.tile([128, 512], f32, tag="pq")
                        for ko in range(8):
                            nc.tensor.matmul(
                                pq[:],
                                w_sb[:, ko, mo * 128:(mo + 1) * 128],
                                xnT[:, ko, no * 512:(no + 1) * 512],
                                start=(ko == 0), stop=(ko == 7))
                        nc.vector.tensor_copy(
                            out=dst[:, mo, no * 512:(no + 1) * 512], in_=pq[:])
            wv_sb = wpool.tile([128, 8, HL * DH], bf16, tag="w")
            nc.sync.dma_start(wv_sb[:], wv_in[:])
            for to in range(16):
                pv = psum.tile([128, 256], f32, tag="pv")
                for ko in range(8):
                    nc.tensor.matmul(pv[:],
                                     xnT[:, ko, to * 128:(to + 1) * 128],
                                     wv_sb[:, ko, :],
                                     start=(ko == 0), stop=(ko == 7))
                nc.vector.tensor_copy(
                    out=v_sb[:, to, :, 0:DH],
                    in_=pv[:].rearrange("p (h d) -> p h d", h=HL))
            nc.vector.tensor_copy(out=v_sb[:, :, :, DH:DH + 1],
                                  in_=nc.const_aps.tensor(1.0, (128, 16, HL, 1), f32))

        # ================= Phase 3: causal attention =================
        with ExitStack() as ph:
            epool = ph.enter_context(tc.tile_pool(name="p3e", bufs=20))
            spool = ph.enter_context(tc.tile_pool(name="p3s", bufs=4))
            ps_s = ph.enter_context(tc.tile_pool(name="p3ps", bufs=4,
                                                 space="PSUM"))
            ps_o = ph.enter_context(tc.tile_pool(name="p3po", bufs=2,
                                                 space="PSUM"))
            for h in range(HL):
                po, ch = (h % 2) * 64, h // 2
                for jt in range(4):
                    pvo = ps_o.tile([128, 512], f32, tag="pvo")
                    n_s = 4 * jt + 4
                    ets = []
                    for it in range(n_s):
                        pss = ps_s.tile([128, 512], f32, tag="pss")
                        nc.tensor.matmul(
                            pss[:],
                            kT[po:po + 64, ch, it * 128:(it + 1) * 128],
                            qT[po:po + 64, ch, jt * 512:(jt + 1) * 512],
                            start=True, stop=True)
                        et = epool.tile([128, 512], bf16, tag="et",
                                        name=f"et{h}_{jt}_{it}")
                        nc.scalar.activation(et[:], pss[:], AF.Exp, scale=0.125)
                        if it >= 4 * jt:
                            nc.vector.tensor_tensor(
                                et[:], et[:], masks[:, it - 4 * jt, :], ALU.mult)
                        ets.append(et)
                    for it in range(n_s):
                        nc.tensor.matmul(pvo[0:DH + 1, :],
                                         v_sb[:, it, h, :], ets[it][:],
                                         start=(it == 0), stop=(it == n_s - 1))
                    lrow = spool.tile([1, 512], f32, tag="lrow")
                    nc.vector.tensor_copy(lrow[:], pvo[DH:DH + 1, :])
                    lb = spool.tile([64, 512], f32, tag="lb")
                    nc.gpsimd.partition_broadcast(lb[:], lrow[:])
                    nc.vector.reciprocal_approx_fast(out=lb[:], in_=lb[:])
                    nc.vector.tensor_tensor(
                        oaT[po:po + 64, ch, jt * 512:(jt + 1) * 512],
                        pvo[0:DH, :], lb[:], ALU.mult)

        # ==== Phase 4: output projection partials + chunked RS ====
        # Chunk k holds, for each group rank r, rows [512r+128k, 512r+128k+128)
        # of the projection partial, so ReduceScatter hands rank r its rows.
        partial_perm = [dram.tile([TG, D], bf16, name=f"pperm{k}")
                        for k in range(4)]
        rs_chunks = [dram.tile([128, D], bf16, name=f"rsc{k}") for k in range(4)]
        with ExitStack() as ph:
            work = ph.enter_context(tc.tile_pool(name="p4work", bufs=3))
            psum = ph.enter_context(tc.tile_pool(name="p4psum", bufs=4,
                                                 space="PSUM"))
            for k in range(4):
                for r in range(4):
                    to = 4 * r + k
                    prt = work.tile([128, D], bf16, tag="prt")
                    for no in range(2):
                        pp = psum.tile([128, 512], f32, tag="pp")
                        for ko in range(2):
                            nc.tensor.matmul(
                                pp[:],
                                oaT[:, ko, to * 128:(to + 1) * 128],
                                wp_sb[:, ko, no * 512:(no + 1) * 512],
                                start=(ko == 0), stop=(ko == 1))
                        nc.vector.tensor_copy(
                            out=prt[:, no * 512:(no + 1) * 512], in_=pp[:])
                    nc.sync.dma_start(
                        partial_perm[k][r * 128:(r + 1) * 128, :], prt[:])
                nc.gpsimd.collective_compute(
                    "ReduceScatter", ALU.add,
                    replica_groups=[[0, 1, 2, 3], [4, 5, 6, 7]],
                    ins=[partial_perm[k][:].opt()],
                    outs=[rs_chunks[k][:].opt()])

        # ======= Phase 6: residual + LN2 + transpose (512 rows) =======
        x2 = mid.tile([128, 4, D], f32, tag="mid")
        xn2T = mid.tile([128, 8, TG], bf16, tag="mid")
        with ExitStack() as ph:
            work = ph.enter_context(tc.tile_pool(name="p6work", bufs=2))
            small = ph.enter_context(tc.tile_pool(name="p6small", bufs=4))
            psum = ph.enter_context(tc.tile_pool(name="p6psum", bufs=6,
                                                 space="PSUM"))
            bp_row = work.tile([1, D], f32, tag="brow", bufs=1)
            nc.sync.dma_start(bp_row[:], bp_in[:])
            bpb = work.tile([128, D], f32, tag="bpb", bufs=1)
            nc.gpsimd.partition_broadcast(bpb[:], bp_row[:])
            b2_row = work.tile([1, D], f32, tag="brow2", bufs=1)
            nc.sync.dma_start(b2_row[:], b2_in[:])
            b2b = work.tile([128, D], f32, tag="b2b", bufs=1)
            nc.gpsimd.partition_broadcast(b2b[:], b2_row[:])
            for t2 in range(4):
                prt = work.tile([128, D], bf16, tag="prt")
                nc.sync.dma_start(prt[:], rs_chunks[t2][:])
                xrt = work.tile([128, D], f32, tag="xrt")
                nc.sync.dma_start(xrt[:], xg_in[t2 * 128:(t2 + 1) * 128, :])
                x2s = x2[:, t2, :]
                nc.vector.tensor_tensor(x2s, prt[:], xrt[:], ALU.add)
                nc.vector.tensor_tensor(x2s, x2s, bpb[:], ALU.add)
                rstd, nmr = _ln_stats(nc, small, x2s, D, epst[:])
                xn2 = work.tile([128, D], bf16, tag="xn2")
                nc.scalar.activation(xn2[:], x2s, AF.Identity,
                                     bias=nmr[:], scale=rstd[:])
                for do in range(8):
                    ptr = psum.tile([128, 128], bf16, tag="ptr")
                    nc.tensor.transpose(ptr[:], xn2[:, do * 128:(do + 1) * 128],
                                        ident[:])
                    nc.scalar.activation(
                        xn2T[:, do, t2 * 128:(t2 + 1) * 128], ptr[:],
                        AF.Identity, bias=be2t[:, do:do + 1],
                        scale=g2t[:, do:do + 1])
                # fold b2 into the residual copy for the FFN epilogue
                nc.vector.tensor_tensor(x2s, x2s, b2b[:], ALU.add)

        # ================= Phase 7: FFN first matmul =================
        hT = persist.tile([128, 32, TG], bf16, tag="big")
        with ExitStack() as ph:
            wpool = ph.enter_context(tc.tile_pool(name="p7w", bufs=8))
            psum = ph.enter_context(tc.tile_pool(name="p7psum", bufs=6,
                                                 space="PSUM"))
            for mo in range(32):
                w1t = wpool.tile([128, 8, 128], bf16, tag="w1t")
                nc.sync.dma_start(w1t[:], w1_in[:, mo])
                ph_ = psum.tile([128, 512], f32, tag="ph")
                for ko in range(8):
                    nc.tensor.matmul(ph_[:], w1t[:, ko, :], xn2T[:, ko, :],
                                     start=(ko == 0), stop=(ko == 7))
                nc.scalar.activation(hT[:, mo, :], ph_[:], AF.Relu,
                                     bias=b1s[:, mo:mo + 1])

        # ============ Phase 8: FFN second matmul + epilogue ============
        with ExitStack() as ph:
            wpool = ph.enter_context(tc.tile_pool(name="p8w", bufs=8))
            work = ph.enter_context(tc.tile_pool(name="p8work", bufs=2))
            psum = ph.enter_context(tc.tile_pool(name="p8psum", bufs=8,
                                                 space="PSUM"))
            py = [psum.tile([128, 512], f32, tag="py", name=f"py{i}")
                  for i in range(8)]
            for ko in range(32):
                w2t = wpool.tile([128, D], bf16, tag="w2t")
                nc.sync.dma_start(w2t[:], w2_in[ko * 128:(ko + 1) * 128, :])
                for m2 in range(4):
                    for no in range(2):
                        nc.tensor.matmul(
                            py[m2 * 2 + no][:],
                            hT[:, ko, m2 * 128:(m2 + 1) * 128],
                            w2t[:, no * 512:(no + 1) * 512],
                            start=(ko == 0), stop=(ko == 31))
            for m2 in range(4):
                osb = work.tile([128, D], f32, tag="osb")
                for no in range(2):
                    nc.vector.tensor_tensor(
                        osb[:, no * 512:(no + 1) * 512],
                        py[m2 * 2 + no][:], x2[:, m2, no * 512:(no + 1) * 512],
                        ALU.add)
                nc.sync.dma_start(out_dram[m2 * 128:(m2 + 1) * 128, :], osb[:])

    nc.compile()
    return nc


def _prep(inputs):
    x = np.asarray(inputs["x"], np.float32)
    wq = np.asarray(inputs["wq"], np.float32)
    wk = np.asarray(inputs["wk"], np.float32)
    wv = np.asarray(inputs["wv"], np.float32)
    wp = np.asarray(inputs["w_proj"], np.float32)
    bp = np.asarray(inputs["b_proj"], np.float32)
    w1 = np.asarray(inputs["w1"], np.float32)
    b1 = np.asarray(inputs["b1"], np.float32)
    w2 = np.asarray(inputs["w2"], np.float32)
    b2 = np.asarray(inputs["b2"], np.float32)
    ln1_g = np.asarray(inputs["ln1_g"], np.float32)
    ln1_b = np.asarray(inputs["ln1_b"], np.float32)
    ln2_g = np.asarray(inputs["ln2_g"], np.float32)
    ln2_b = np.asarray(inputs["ln2_b"], np.float32)

    w1r = np.ascontiguousarray(
        w1.reshape(8, 128, 32, 128).transpose(1, 2, 0, 3)).astype(BF16)
    w2r = w2.astype(BF16)
    b1r = np.ascontiguousarray(b1.reshape(32, 128).T)
    ln_tiles = {
        "ln1g": np.ascontiguousarray(ln1_g.reshape(8, 128).T),
        "ln1b": np.ascontiguousarray(ln1_b.reshape(8, 128).T),
        "ln2g": np.ascontiguousarray(ln2_g.reshape(8, 128).T),
        "ln2b": np.ascontiguousarray(ln2_b.reshape(8, 128).T),
    }
    in_maps = []
    for c in range(N_CORES):
        b, g = divmod(c, 4)
        h0 = HL * g
        wqc = np.concatenate([wq[h] for h in range(h0, h0 + HL)], axis=1)
        wkc = np.concatenate([wk[h] for h in range(h0, h0 + HL)], axis=1)
        wvc = np.concatenate([wv[h] for h in range(h0, h0 + HL)], axis=1)
        wpc = wp[h0 * DH:(h0 + HL) * DH, :]
        in_maps.append({
            "x": np.ascontiguousarray(x[b]),
            "xg": np.ascontiguousarray(x[b, g * TG:(g + 1) * TG, :]),
            "wq": np.ascontiguousarray(
                wqc.reshape(8, 128, HL * DH).transpose(1, 0, 2)).astype(BF16),
            "wk": np.ascontiguousarray(
                wkc.reshape(8, 128, HL * DH).transpose(1, 0, 2)).astype(BF16),
            "wv": np.ascontiguousarray(
                wvc.reshape(8, 128, HL * DH).transpose(1, 0, 2)).astype(BF16),
            "wp": np.ascontiguousarray(
                wpc.reshape(2, 128, D).transpose(1, 0, 2)).astype(BF16),
            "bp": np.ascontiguousarray(bp.reshape(1, D)),
            "w1": w1r,
            "b1": b1r,
            "w2": w2r,
            "b2": np.ascontiguousarray(b2.reshape(1, D)),
            **ln_tiles,
        })
    return in_maps


def _make_runner(nc):
    """Build a cached jitted SPMD executor (mirrors bass2jax.run_bass_via_pjrt
    but jits once and is reused across kernel() calls)."""
    import jax
    from jax.experimental.shard_map import shard_map
    from jax.sharding import Mesh, PartitionSpec
    from concourse import bass2jax as b2j

    b2j.install_neuronx_cc_hook()
    partition_name = (nc.partition_id_tensor.name
                      if nc.partition_id_tensor else None)
    in_names, out_names, out_avals, zero_shapes = [], [], [], []
    for alloc in nc.m.functions[0].allocations:
        if not isinstance(alloc, mybir.MemoryLocationSet):
            continue
        name = alloc.memorylocations[0].name
        if alloc.kind == "ExternalInput":
            if name != partition_name:
                in_names.append(name)
        elif alloc.kind == "ExternalOutput":
            shape = tuple(alloc.tensor_shape)
            dtype = mybir.dt.np(alloc.dtype)
            out_names.append(name)
            out_avals.append(jax.core.ShapedArray(shape, dtype))
            zero_shapes.append((shape, dtype))
    n_params = len(in_names)
    n_outs = len(out_avals)
    all_in_names = list(in_names) + list(out_names)
    if partition_name is not None:
        all_in_names.append(partition_name)
    donate = tuple(range(n_params, n_params + n_outs))

    def _body(*args):
        operands = list(args)
        if partition_name is not None:
            operands.append(b2j.partition_id_tensor())
        outs = b2j._bass_exec_p.bind(
            *operands,
            out_avals=tuple(out_avals),
            in_names=tuple(all_in_names),
            out_names=tuple(out_names),
            lowering_input_output_aliases=(),
            sim_require_finite=True,
            sim_require_nnan=True,
            nc=nc,
        )
        return tuple(outs)

    devices = jax.devices()[:N_CORES]
    mesh = Mesh(np.asarray(devices), ("core",))
    in_specs = (PartitionSpec("core"),) * (n_params + n_outs)
    out_specs = (PartitionSpec("core"),) * n_outs
    sharded = jax.jit(
        shard_map(_body, mesh=mesh, in_specs=in_specs, out_specs=out_specs,
                  check_rep=False),
        donate_argnums=donate, keep_unused=True)

    def run(in_maps):
        concat_in = [
            np.concatenate([np.asarray(in_maps[c][name])
                            for c in range(N_CORES)], axis=0)
            for name in in_names
        ]
        concat_zeros = [
            np.zeros((N_CORES * s[0], *s[1:]), dt) for s, dt in zero_shapes
        ]
        out_arrs = sharded(*concat_in, *concat_zeros)
        return [
            {name: np.asarray(out_arrs[i]).reshape(N_CORES,
                                                   *zero_shapes[i][0])[c]
             for i, name in enumerate(out_names)}
            for c in range(N_CORES)
        ]

    return run


_CACHE = {}


def _get_nc():
    if "nc" not in _CACHE:
        _CACHE["nc"] = _build()
    return _CACHE["nc"]


def _get_runner():
    if "run" not in _CACHE:
        _CACHE["run"] = _make_runner(_get_nc())
    return _CACHE["run"]


def kernel(**inputs):
    run = _get_runner()
    in_maps = _prep(inputs)
    res = run(in_maps)
    B = 2
    out = np.empty((B, T, D), np.float32)
    for c in range(N_CORES):
        b, g = divmod(c, 4)
        out[b, g * TG:(g + 1) * TG, :] = res[c]["out"]
    return out


# revision 13
# speedup vs baseline: 1.3480x; 1.1803x over previous
"""Distributed Trainium2 (Bass/Tile) kernel for a pre-LN transformer block.

Reference computation (per batch element):
    xn = LN1(x); q,k,v = per-head projections of xn
    attn = causal-softmax(q k^T / sqrt(dh)) v
    x1 = x + concat_heads(attn) @ w_proj + b_proj
    out = x1 + relu(LN2(x1) @ w1 + b1) @ w2 + b2

Sharding over 8 NeuronCores: core c handles batch b=c//4 and head group
g=c%4 (4 of 16 heads).  Attention + projection partials are head-parallel;
a ReduceScatter(add) over each 4-core group turns the projection partials
into per-core 512-row slices of x1; the FFN then runs sequence-parallel
(512 rows per core) with no further communication.  The host assembles the
8 [512,1024] outputs into the full [2,2048,1024] result.

Implementation notes:
- Matmul operands are bf16 (fast weight loads, fp32 PSUM accumulation).
- LayerNorm gains/biases are folded into the adjacent weight matrices on
  the host, so the device applies only (x-mu)*rstd.
- Activation transposes use the DMA XBAR (bf16), not the PE array.
- The pipeline is window-ordered: for each 512-column window of the
  sequence, QKV projections, attention, and the output projection are
  emitted together so the tensor engine stays busy.
- A chain of throwaway matmuls keeps the PE clock warm during the
  collective.
"""

import numpy as np
import ml_dtypes

import concourse.bass as bass
import concourse.mybir as mybir
import concourse.tile as tile
from contextlib import ExitStack
from concourse import bacc
from concourse.bass_utils import run_bass_kernel_spmd

T = 2048          # sequence length
D = 1024          # embedding dim
H = 16            # total heads
DH = 64           # head dim
HL = 4            # heads per core
TG = 512          # rows per core in the FFN phase
DF = 4096         # FFN hidden dim
EPS = 1e-5
N_CORES = 8
N_WARM = 110      # PE warm-keeper matmuls during the collective

f32 = mybir.dt.float32
bf16 = mybir.dt.bfloat16
AF = mybir.ActivationFunctionType
ALU = mybir.AluOpType
BF16 = ml_dtypes.bfloat16


def _ln_stats(nc, pool, xt, width, eps_ap):
    """Per-partition mean/var over `width` free elements -> (rstd, neg_mu_rstd)."""
    nchunk = width // 512
    bns = pool.tile([128, nchunk, 6], f32, tag="bns")
    for i in range(nchunk):
        nc.vector.bn_stats(bns[:, i, :], xt[:, i * 512:(i + 1) * 512])
    agg = pool.tile([128, 2], f32, tag="agg")
    nc.vector.bn_aggr(agg[:], bns[:].rearrange("p a b -> p (a b)"))
    std = pool.tile([128, 1], f32, tag="std")
    nc.scalar.activation(std[:], agg[:, 1:2], AF.Sqrt, bias=eps_ap)
    rstd = pool.tile([128, 1], f32, tag="rstd")
    nc.vector.reciprocal(rstd[:], std[:])
    nmr = pool.tile([128, 1], f32, tag="nmr")
    nc.vector.tensor_scalar(nmr[:], agg[:, 0:1], rstd[:], -1.0, ALU.mult, ALU.mult)
    return rstd, nmr


def _build():
    nc = bacc.Bacc("TRN2", target_bir_lowering=False, debug=False,
                   num_devices=N_CORES)

    x_in = nc.dram_tensor("x", [T, D], f32, kind="ExternalInput")
    wq_in = nc.dram_tensor("wq", [128, 8, HL * DH], bf16, kind="ExternalInput")
    wk_in = nc.dram_tensor("wk", [128, 8, HL * DH], bf16, kind="ExternalInput")
    wv_in = nc.dram_tensor("wv", [128, 8, HL * DH], bf16, kind="ExternalInput")
    qb_in = nc.dram_tensor("qb", [128, 2], f32, kind="ExternalInput")
    kb_in = nc.dram_tensor("kb", [128, 2], f32, kind="ExternalInput")
    vb_in = nc.dram_tensor("vb", [1, HL * DH], f32, kind="ExternalInput")
    wp_in = nc.dram_tensor("wp", [128, 2, D], bf16, kind="ExternalInput")
    bp_in = nc.dram_tensor("bp", [1, D], f32, kind="ExternalInput")
    w1_in = nc.dram_tensor("w1", [128, 32, 8, 128], bf16, kind="ExternalInput")
    b1_in = nc.dram_tensor("b1", [128, 32], f32, kind="ExternalInput")
    w2_in = nc.dram_tensor("w2", [DF, D], bf16, kind="ExternalInput")
    b2_in = nc.dram_tensor("b2", [1, D], f32, kind="ExternalInput")
    xg_in = nc.dram_tensor("xg", [TG, D], f32, kind="ExternalInput")
    out_dram = nc.dram_tensor("out", [TG, D], f32, kind="ExternalOutput")

    with tile.TileContext(nc) as tc, ExitStack() as top:
        persist = top.enter_context(tc.tile_pool(name="persist", bufs=1))
        mid = top.enter_context(tc.tile_pool(name="mid", bufs=4))
        consts = top.enter_context(tc.tile_pool(name="consts", bufs=1))
        dram = top.enter_context(tc.tile_pool(name="dram", bufs=1, space="DRAM"))

        # ---- constants ----
        masks = consts.tile([128, HL, 512], bf16, tag="masks")
        nc.vector.memset(masks[:], 1.0)
        for d in range(HL):
            # keep (1.0) where global_t - global_s >= 0, i.e. f - p - 128*d >= 0
            nc.gpsimd.affine_select(
                out=masks[:, d, :], in_=masks[:, d, :],
                compare_op=ALU.is_ge, fill=0.0,
                base=-128 * d, pattern=[[1, 512]], channel_multiplier=-1)
        epst = consts.tile([128, 1], f32, tag="epst")
        nc.vector.memset(epst[:], EPS)
        b1s = consts.tile([128, 32], f32, tag="b1s")
        nc.sync.dma_start(b1s[:], b1_in[:])
        qbt = consts.tile([128, 2], f32, tag="qbt")
        nc.sync.dma_start(qbt[:], qb_in[:])
        kbt = consts.tile([128, 2], f32, tag="kbt")
        nc.sync.dma_start(kbt[:], kb_in[:])
        vbr = consts.tile([1, HL * DH], f32, tag="vbr")
        nc.sync.dma_start(vbr[:], vb_in[:])
        vbb = consts.tile([128, HL, DH], f32, tag="vbb")
        nc.gpsimd.partition_broadcast(vbb[:].rearrange("p h d -> p (h d)"), vbr[:])
        wp_sb = consts.tile([128, 2, D], bf16, tag="wp_sb")
        nc.sync.dma_start(wp_sb[:], wp_in[:])
        wq_sb = consts.tile([128, 8, HL * DH], bf16, tag="wq_sb")
        nc.sync.dma_start(wq_sb[:], wq_in[:])
        wk_sb = consts.tile([128, 8, HL * DH], bf16, tag="wk_sb")
        nc.sync.dma_start(wk_sb[:], wk_in[:])
        wv_sb = consts.tile([128, 8, HL * DH], bf16, tag="wv_sb")
        nc.sync.dma_start(wv_sb[:], wv_in[:])

        # ---- persistent activation tiles ----
        xn_all = persist.tile([128, 16, D], bf16, tag="bigA")   # LN1(x), t-major
        xnT = persist.tile([128, 8, T], bf16, tag="bigB")       # LN1(x)^T
        qT = mid.tile([128, 2, T], bf16, tag="mid")             # q^T (4 heads)
        kT = mid.tile([128, 2, T], bf16, tag="mid")             # k^T
        v_sb = mid.tile([128, 16, HL, DH + 1], bf16, tag="mid")  # v + ones col
        oaT = mid.tile([128, 2, T], bf16, tag="mid")            # attn out^T

        # ================= Phase 1: LN1 + DMA transpose =================
        with ExitStack() as ph:
            work = ph.enter_context(tc.tile_pool(name="p1work", bufs=3))
            small = ph.enter_context(tc.tile_pool(name="p1small", bufs=4))
            for to in range(16):
                xt = work.tile([128, D], f32, tag="xt")
                nc.sync.dma_start(xt[:], x_in[to * 128:(to + 1) * 128, :])
                rstd, nmr = _ln_stats(nc, small, xt, D, epst[:])
                nc.scalar.activation(xn_all[:, to, :], xt[:], AF.Identity,
                                     bias=nmr[:], scale=rstd[:])
            for to in range(16):
                nc.sync.dma_start_transpose(
                    xnT[:, :, to * 128:(to + 1) * 128], xn_all[:, to, :])

        # ========= Phase 2-4: windowed QKV + attention + projection =========
        partial_dram = dram.tile([T, D], bf16)
        rs_out = dram.tile([TG, D], bf16)
        last_prt = None
        with ExitStack() as ph:
            epool = ph.enter_context(tc.tile_pool(name="p3e", bufs=20))
            spool = ph.enter_context(tc.tile_pool(name="p3s", bufs=4))
            work = ph.enter_context(tc.tile_pool(name="p4work", bufs=3))
            ps_q = ph.enter_context(tc.tile_pool(name="psq", bufs=2, space="PSUM"))
            ps_s = ph.enter_context(tc.tile_pool(name="pss", bufs=3, space="PSUM"))
            ps_o = ph.enter_context(tc.tile_pool(name="pso", bufs=2, space="PSUM"))
            ps_p = ph.enter_context(tc.tile_pool(name="psp", bufs=1, space="PSUM"))
            for w in range(4):
                # q^T and k^T for this 512-column window
                for dst, w_sb, bias in ((qT, wq_sb, qbt), (kT, wk_sb, kbt)):
                    for mo in range(2):
                        pq = ps_q.tile([128, 512], f32, tag="pq")
                        for ko in range(8):
                            nc.tensor.matmul(
                                pq[:],
                                w_sb[:, ko, mo * 128:(mo + 1) * 128],
                                xnT[:, ko, w * 512:(w + 1) * 512],
                                start=(ko == 0), stop=(ko == 7))
                        nc.vector.tensor_scalar(
                            dst[:, mo, w * 512:(w + 1) * 512], pq[:],
                            bias[:, mo:mo + 1], None, ALU.add)
                # v rows for this window's four 128-row chunks
                for to in range(4 * w, 4 * w + 4):
                    pv = ps_q.tile([128, 512], f32, tag="pq")
                    for ko in range(8):
                        nc.tensor.matmul(pv[:, 0:256],
                                         xnT[:, ko, to * 128:(to + 1) * 128],
                                         wv_sb[:, ko, :],
                                         start=(ko == 0), stop=(ko == 7))
                    nc.vector.tensor_tensor(
                        v_sb[:, to, :, 0:DH],
                        pv[:, 0:256].rearrange("p (h d) -> p h d", h=HL),
                        vbb[:], ALU.add)
                if w == 0:
                    nc.vector.tensor_copy(
                        out=v_sb[:, :, :, DH:DH + 1],
                        in_=nc.const_aps.tensor(1.0, (128, 16, HL, 1), f32))
                # causal attention for jt = w, all four heads
                for h in range(HL):
                    po, ch = (h % 2) * 64, h // 2
                    pvo = ps_o.tile([128, 512], f32, tag="pvo")
                    n_s = 4 * w + 4
                    ets = []
                    for it in range(n_s):
                        pss = ps_s.tile([128, 512], f32, tag="pss")
                        nc.tensor.matmul(
                            pss[:],
                            kT[po:po + 64, ch, it * 128:(it + 1) * 128],
                            qT[po:po + 64, ch, w * 512:(w + 1) * 512],
                            start=True, stop=True)
                        et = epool.tile([128, 512], bf16, tag="et",
                                        name=f"et{h}_{w}_{it}")
                        nc.scalar.activation(et[:], pss[:], AF.Exp, scale=0.125)
                        if it >= 4 * w:
                            nc.vector.tensor_tensor(
                                et[:], et[:], masks[:, it - 4 * w, :], ALU.mult)
                        ets.append(et)
                    for it in range(n_s):
                        nc.tensor.matmul(pvo[0:DH + 1, :],
                                         v_sb[:, it, h, :], ets[it][:],
                                         start=(it == 0), stop=(it == n_s - 1))
                    lrow = spool.tile([1, 512], f32, tag="lrow")
                    nc.vector.tensor_copy(lrow[:], pvo[DH:DH + 1, :])
                    lb = spool.tile([64, 512], f32, tag="lb")
                    nc.gpsimd.partition_broadcast(lb[:], lrow[:])
                    nc.vector.reciprocal_approx_fast(out=lb[:], in_=lb[:])
                    nc.vector.tensor_tensor(
                        oaT[po:po + 64, ch, w * 512:(w + 1) * 512],
                        pvo[0:DH, :], lb[:], ALU.mult)
                # output projection partial rows for this window
                for to in range(4 * w, 4 * w + 4):
                    prt = work.tile([128, D], bf16, tag="prt")
                    for no in range(2):
                        pp = ps_p.tile([128, 512], f32, tag="pp")
                        for ko in range(2):
                            nc.tensor.matmul(
                                pp[:],
                                oaT[:, ko, to * 128:(to + 1) * 128],
                                wp_sb[:, ko, no * 512:(no + 1) * 512],
                                start=(ko == 0), stop=(ko == 1))
                        nc.vector.tensor_copy(
                            out=prt[:, no * 512:(no + 1) * 512], in_=pp[:])
                    nc.sync.dma_start(
                        partial_dram[to * 128:(to + 1) * 128, :], prt[:])
                    last_prt = prt

            # PE warm-keeper chain: throwaway matmuls that execute while the
            # ReduceScatter is in flight, so the PE clock does not drop.
            warm_scratch = dram.tile([128, 512], f32)
            wsb = spool.tile([128, 512], f32, tag="wsb")
            for dd in range(N_WARM):
                pd = ps_q.tile([128, 512], f32, tag="pq", name=f"warm{dd}")
                nc.tensor.matmul(pd[:], last_prt[:, 0:128], last_prt[:, 0:512],
                                 start=True, stop=True)
                if dd == N_WARM - 1:
                    nc.vector.tensor_copy(wsb[:], pd[:])
            nc.sync.dma_start(warm_scratch[:], wsb[:])

        # ================= Phase 5: ReduceScatter =================
        nc.gpsimd.collective_compute(
            "ReduceScatter", ALU.add,
            replica_groups=[[0, 1, 2, 3], [4, 5, 6, 7]],
            ins=[partial_dram[:].opt()],
            outs=[rs_out[:].opt()])

        # ======= Phase 6: residual + LN2 + DMA transpose (512 rows) =======
        x2 = mid.tile([128, 4, D], bf16, tag="mid")
        xn2T = mid.tile([128, 8, TG], bf16, tag="mid")
        with ExitStack() as ph:
            work = ph.enter_context(tc.tile_pool(name="p6work", bufs=2))
            small = ph.enter_context(tc.tile_pool(name="p6small", bufs=4))
            bp_row = work.tile([1, D], f32, tag="brow", bufs=1)
            nc.sync.dma_start(bp_row[:], bp_in[:])
            bpb = work.tile([128, D], f32, tag="bpb", bufs=1)
            nc.gpsimd.partition_broadcast(bpb[:], bp_row[:])
            b2_row = work.tile([1, D], f32, tag="brow2", bufs=1)
            nc.sync.dma_start(b2_row[:], b2_in[:])
            b2b = work.tile([128, D], f32, tag="b2b", bufs=1)
            nc.gpsimd.partition_broadcast(b2b[:], b2_row[:])
            xn2_all = work.tile([128, 4, D], bf16, tag="xn2a", bufs=1)
            for t2 in range(4):
                prt = work.tile([128, D], bf16, tag="prt")
                nc.sync.dma_start(prt[:], rs_out[t2 * 128:(t2 + 1) * 128, :])
                xrt = work.tile([128, D], f32, tag="xrt")
                nc.sync.dma_start(xrt[:], xg_in[t2 * 128:(t2 + 1) * 128, :])
                x2f = work.tile([128, D], f32, tag="x2f")
                nc.vector.tensor_tensor(x2f[:], prt[:], xrt[:], ALU.add)
                nc.vector.tensor_tensor(x2f[:], x2f[:], bpb[:], ALU.add)
                rstd, nmr = _ln_stats(nc, small, x2f[:], D, epst[:])
                nc.scalar.activation(xn2_all[:, t2, :], x2f[:], AF.Identity,
                                     bias=nmr[:], scale=rstd[:])
                # fold b2 into the residual copy for the FFN epilogue
                nc.vector.tensor_tensor(x2[:, t2, :], x2f[:], b2b[:], ALU.add)
            for t2 in range(4):
                nc.sync.dma_start_transpose(
                    xn2T[:, :, t2 * 128:(t2 + 1) * 128], xn2_all[:, t2, :])

        # ================= Phase 7: FFN first matmul =================
        hT = persist.tile([128, 32, TG], bf16, tag="bigA")
        with ExitStack() as ph:
            wpool = ph.enter_context(tc.tile_pool(name="p7w", bufs=8))
            psum = ph.enter_context(tc.tile_pool(name="p7psum", bufs=6,
                                                 space="PSUM"))
            for mo in range(32):
                w1t = wpool.tile([128, 8, 128], bf16, tag="w1t")
                nc.sync.dma_start(w1t[:], w1_in[:, mo])
                ph_ = psum.tile([128, 512], f32, tag="ph")
                for ko in range(8):
                    nc.tensor.matmul(ph_[:], w1t[:, ko, :], xn2T[:, ko, :],
                                     start=(ko == 0), stop=(ko == 7))
                nc.scalar.activation(hT[:, mo, :], ph_[:], AF.Relu,
                                     bias=b1s[:, mo:mo + 1])

        # ============ Phase 8: FFN second matmul + epilogue ============
        with ExitStack() as ph:
            wpool = ph.enter_context(tc.tile_pool(name="p8w", bufs=8))
            work = ph.enter_context(tc.tile_pool(name="p8work", bufs=2))
            psum = ph.enter_context(tc.tile_pool(name="p8psum", bufs=8,
                                                 space="PSUM"))
            py = [psum.tile([128, 512], f32, tag="py", name=f"py{i}")
                  for i in range(8)]
            for ko in range(32):
                w2t = wpool.tile([128, D], bf16, tag="w2t")
                nc.sync.dma_start(w2t[:], w2_in[ko * 128:(ko + 1) * 128, :])
                for m2 in range(4):
                    for no in range(2):
                        nc.tensor.matmul(
                            py[m2 * 2 + no][:],
                            hT[:, ko, m2 * 128:(m2 + 1) * 128],
                            w2t[:, no * 512:(no + 1) * 512],
                            start=(ko == 0), stop=(ko == 31))
            for m2 in range(4):
                osb = work.tile([128, D], f32, tag="osb")
                for no in range(2):
                    nc.vector.tensor_tensor(
                        osb[:, no * 512:(no + 1) * 512],
                        py[m2 * 2 + no][:], x2[:, m2, no * 512:(no + 1) * 512],
                        ALU.add)
                nc.sync.dma_start(out_dram[m2 * 128:(m2 + 1) * 128, :], osb[:])

    nc.compile()
    return nc


def _prep(inputs):
    x = np.asarray(inputs["x"], np.float32)
    wq = np.asarray(inputs["wq"], np.float32)
    wk = np.asarray(inputs["wk"], np.float32)
    wv = np.asarray(inputs["wv"], np.float32)
    wp = np.asarray(inputs["w_proj"], np.float32)
    bp = np.asarray(inputs["b_proj"], np.float32)
    w1 = np.asarray(inputs["w1"], np.float32)
    b1 = np.asarray(inputs["b1"], np.float32)
    w2 = np.asarray(inputs["w2"], np.float32)
    b2 = np.asarray(inputs["b2"], np.float32)
    ln1_g = np.asarray(inputs["ln1_g"], np.float32)
    ln1_b = np.asarray(inputs["ln1_b"], np.float32)
    ln2_g = np.asarray(inputs["ln2_g"], np.float32)
    ln2_b = np.asarray(inputs["ln2_b"], np.float32)

    # fold LN gains into the adjacent weights (host-side)
    w1f = ln2_g[:, None] * w1                     # [1024, 4096]
    b1f = b1 + ln2_b @ w1                         # [4096]
    w1r = np.ascontiguousarray(
        w1f.reshape(8, 128, 32, 128).transpose(1, 2, 0, 3)).astype(BF16)
    w2r = w2.astype(BF16)
    b1r = np.ascontiguousarray(b1f.reshape(32, 128).T)

    in_maps = []
    for c in range(N_CORES):
        b, g = divmod(c, 4)
        h0 = HL * g
        wqc = np.concatenate([wq[h] for h in range(h0, h0 + HL)], axis=1)
        wkc = np.concatenate([wk[h] for h in range(h0, h0 + HL)], axis=1)
        wvc = np.concatenate([wv[h] for h in range(h0, h0 + HL)], axis=1)
        qb = ln1_b @ wqc                          # [256]
        kb = ln1_b @ wkc
        vb = ln1_b @ wvc
        wqf = ln1_g[:, None] * wqc
        wkf = ln1_g[:, None] * wkc
        wvf = ln1_g[:, None] * wvc
        wpc = wp[h0 * DH:(h0 + HL) * DH, :]
        in_maps.append({
            "x": np.ascontiguousarray(x[b]),
            "xg": np.ascontiguousarray(x[b, g * TG:(g + 1) * TG, :]),
            "wq": np.ascontiguousarray(
                wqf.reshape(8, 128, HL * DH).transpose(1, 0, 2)).astype(BF16),
            "wk": np.ascontiguousarray(
                wkf.reshape(8, 128, HL * DH).transpose(1, 0, 2)).astype(BF16),
            "wv": np.ascontiguousarray(
                wvf.reshape(8, 128, HL * DH).transpose(1, 0, 2)).astype(BF16),
            "qb": np.ascontiguousarray(qb.reshape(2, 128).T),
            "kb": np.ascontiguousarray(kb.reshape(2, 128).T),
            "vb": np.ascontiguousarray(vb.reshape(1, HL * DH)),
            "wp": np.ascontiguousarray(
                wpc.reshape(2, 128, D).transpose(1, 0, 2)).astype(BF16),
            "bp": np.ascontiguousarray(bp.reshape(1, D)),
            "w1": w1r,
            "b1": b1r,
            "w2": w2r,
            "b2": np.ascontiguousarray(b2.reshape(1, D)),
        })
    return in_maps


def _make_runner(nc):
    """Build a cached jitted SPMD executor (mirrors bass2jax.run_bass_via_pjrt
    but jits once and is reused across kernel() calls)."""
    import jax
    from jax.experimental.shard_map import shard_map
    from jax.sharding import Mesh, PartitionSpec
    from concourse import bass2jax as b2j

    b2j.install_neuronx_cc_hook()
    partition_name = (nc.partition_id_tensor.name
                      if nc.partition_id_tensor else None)
    in_names, out_names, out_avals, zero_shapes = [], [], [], []
    for alloc in nc.m.functions[0].allocations:
        if not isinstance(alloc, mybir.MemoryLocationSet):
            continue
        name = alloc.memorylocations[0].name
        if alloc.kind == "ExternalInput":
            if name != partition_name:
                in_names.append(name)
        elif alloc.kind == "ExternalOutput":
            shape = tuple(alloc.tensor_shape)
            dtype = mybir.dt.np(alloc.dtype)
            out_names.append(name)
            out_avals.append(jax.core.ShapedArray(shape, dtype))
            zero_shapes.append((shape, dtype))
    n_params = len(in_names)
    n_outs = len(out_avals)
    all_in_names = list(in_names) + list(out_names)
    if partition_name is not None:
        all_in_names.append(partition_name)
    donate = tuple(range(n_params, n_params + n_outs))

    def _body(*args):
        operands = list(args)
        if partition_name is not None:
            operands.append(b2j.partition_id_tensor())
        outs = b2j._bass_exec_p.bind(
            *operands,
            out_avals=tuple(out_avals),
            in_names=tuple(all_in_names),
            out_names=tuple(out_names),
            lowering_input_output_aliases=(),
            sim_require_finite=True,
            sim_require_nnan=True,
            nc=nc,
        )
        return tuple(outs)

    devices = jax.devices()[:N_CORES]
    mesh = Mesh(np.asarray(devices), ("core",))
    in_specs = (PartitionSpec("core"),) * (n_params + n_outs)
    out_specs = (PartitionSpec("core"),) * n_outs
    sharded = jax.jit(
        shard_map(_body, mesh=mesh, in_specs=in_specs, out_specs=out_specs,
                  check_rep=False),
        donate_argnums=donate, keep_unused=True)

    def run(in_maps):
        concat_in = [
            np.concatenate([np.asarray(in_maps[c][name])
                            for c in range(N_CORES)], axis=0)
            for name in in_names
        ]
        concat_zeros = [
            np.zeros((N_CORES * s[0], *s[1:]), dt) for s, dt in zero_shapes
        ]
        out_arrs = sharded(*concat_in, *concat_zeros)
        return [
            {name: np.asarray(out_arrs[i]).reshape(N_CORES,
                                                   *zero_shapes[i][0])[c]
             for i, name in enumerate(out_names)}
            for c in range(N_CORES)
        ]

    return run


_CACHE = {}


def _get_nc():
    if "nc" not in _CACHE:
        _CACHE["nc"] = _build()
    return _CACHE["nc"]


def _get_runner():
    if "run" not in _CACHE:
        _CACHE["run"] = _make_runner(_get_nc())
    return _CACHE["run"]


def kernel(**inputs):
    run = _get_runner()
    in_maps = _prep(inputs)
    res = run(in_maps)
    B = 2
    out = np.empty((B, T, D), np.float32)
    for c in range(N_CORES):
        b, g = divmod(c, 4)
        out[b, g * TG:(g + 1) * TG, :] = res[c]["out"]
    return out


# revision 14
# speedup vs baseline: 1.3566x; 1.0064x over previous
"""Distributed Trainium2 (Bass/Tile) kernel for a pre-LN transformer block.

Reference computation (per batch element):
    xn = LN1(x); q,k,v = per-head projections of xn
    attn = causal-softmax(q k^T / sqrt(dh)) v
    x1 = x + concat_heads(attn) @ w_proj + b_proj
    out = x1 + relu(LN2(x1) @ w1 + b1) @ w2 + b2

Sharding over 8 NeuronCores: core c handles batch b=c//4 and head group
g=c%4 (4 of 16 heads).  Attention + projection partials are head-parallel;
a ReduceScatter(add) over each 4-core group turns the projection partials
into per-core 512-row slices of x1; the FFN then runs sequence-parallel
(512 rows per core) with no further communication.  The host assembles the
8 [512,1024] outputs into the full [2,2048,1024] result.

Implementation notes:
- Matmul operands are bf16 (fast weight loads, fp32 PSUM accumulation).
- LayerNorm gains/biases are folded into the adjacent weight matrices on
  the host, so the device applies only (x-mu)*rstd.
- Activation transposes use the DMA XBAR (bf16), not the PE array.
- The pipeline is window-ordered: for each 512-column window of the
  sequence, QKV projections, attention, and the output projection are
  emitted together so the tensor engine stays busy.
- A chain of throwaway matmuls keeps the PE clock warm during the
  collective.
"""

import numpy as np
import ml_dtypes

import concourse.bass as bass
import concourse.mybir as mybir
import concourse.tile as tile
from contextlib import ExitStack
from concourse import bacc
from concourse.bass_utils import run_bass_kernel_spmd

T = 2048          # sequence length
D = 1024          # embedding dim
H = 16            # total heads
DH = 64           # head dim
HL = 4            # heads per core
TG = 512          # rows per core in the FFN phase
DF = 4096         # FFN hidden dim
EPS = 1e-5
N_CORES = 8
N_WARM = 180      # PE warm-keeper matmuls during the collective

f32 = mybir.dt.float32
bf16 = mybir.dt.bfloat16
AF = mybir.ActivationFunctionType
ALU = mybir.AluOpType
BF16 = ml_dtypes.bfloat16


def _ln_stats(nc, pool, xt, width, eps_ap):
    """Per-partition mean/var over `width` free elements -> (rstd, neg_mu_rstd)."""
    nchunk = width // 512
    bns = pool.tile([128, nchunk, 6], f32, tag="bns")
    for i in range(nchunk):
        nc.vector.bn_stats(bns[:, i, :], xt[:, i * 512:(i + 1) * 512])
    agg = pool.tile([128, 2], f32, tag="agg")
    nc.vector.bn_aggr(agg[:], bns[:].rearrange("p a b -> p (a b)"))
    std = pool.tile([128, 1], f32, tag="std")
    nc.scalar.activation(std[:], agg[:, 1:2], AF.Sqrt, bias=eps_ap)
    rstd = pool.tile([128, 1], f32, tag="rstd")
    nc.vector.reciprocal(rstd[:], std[:])
    nmr = pool.tile([128, 1], f32, tag="nmr")
    nc.vector.tensor_scalar(nmr[:], agg[:, 0:1], rstd[:], -1.0, ALU.mult, ALU.mult)
    return rstd, nmr


def _build():
    nc = bacc.Bacc("TRN2", target_bir_lowering=False, debug=False,
                   num_devices=N_CORES)

    x_in = nc.dram_tensor("x", [T, D], f32, kind="ExternalInput")
    wq_in = nc.dram_tensor("wq", [128, 8, HL * DH], bf16, kind="ExternalInput")
    wk_in = nc.dram_tensor("wk", [128, 8, HL * DH], bf16, kind="ExternalInput")
    wv_in = nc.dram_tensor("wv", [128, 8, HL * DH], bf16, kind="ExternalInput")
    qb_in = nc.dram_tensor("qb", [128, 2], f32, kind="ExternalInput")
    kb_in = nc.dram_tensor("kb", [128, 2], f32, kind="ExternalInput")
    vb_in = nc.dram_tensor("vb", [1, HL * DH], f32, kind="ExternalInput")
    wp_in = nc.dram_tensor("wp", [128, 2, D], bf16, kind="ExternalInput")
    bp_in = nc.dram_tensor("bp", [1, D], f32, kind="ExternalInput")
    w1_in = nc.dram_tensor("w1", [128, 32, 8, 128], bf16, kind="ExternalInput")
    b1_in = nc.dram_tensor("b1", [128, 32], f32, kind="ExternalInput")
    w2_in = nc.dram_tensor("w2", [DF, D], bf16, kind="ExternalInput")
    b2_in = nc.dram_tensor("b2", [1, D], f32, kind="ExternalInput")
    xg_in = nc.dram_tensor("xg", [TG, D], f32, kind="ExternalInput")
    out_dram = nc.dram_tensor("out", [TG, D], f32, kind="ExternalOutput")

    with tile.TileContext(nc) as tc, ExitStack() as top:
        persist = top.enter_context(tc.tile_pool(name="persist", bufs=1))
        mid = top.enter_context(tc.tile_pool(name="mid", bufs=4))
        consts = top.enter_context(tc.tile_pool(name="consts", bufs=1))
        dram = top.enter_context(tc.tile_pool(name="dram", bufs=1, space="DRAM"))

        # ---- constants ----
        masks = consts.tile([128, HL, 512], bf16, tag="masks")
        nc.vector.memset(masks[:], 1.0)
        for d in range(HL):
            # keep (1.0) where global_t - global_s >= 0, i.e. f - p - 128*d >= 0
            nc.gpsimd.affine_select(
                out=masks[:, d, :], in_=masks[:, d, :],
                compare_op=ALU.is_ge, fill=0.0,
                base=-128 * d, pattern=[[1, 512]], channel_multiplier=-1)
        epst = consts.tile([128, 1], f32, tag="epst")
        nc.vector.memset(epst[:], EPS)
        b1s = consts.tile([128, 32], f32, tag="b1s")
        nc.sync.dma_start(b1s[:], b1_in[:])
        qbt = consts.tile([128, 2], f32, tag="qbt")
        nc.sync.dma_start(qbt[:], qb_in[:])
        kbt = consts.tile([128, 2], f32, tag="kbt")
        nc.sync.dma_start(kbt[:], kb_in[:])
        vbr = consts.tile([1, HL * DH], f32, tag="vbr")
        nc.sync.dma_start(vbr[:], vb_in[:])
        vbb = consts.tile([128, HL, DH], f32, tag="vbb")
        nc.gpsimd.partition_broadcast(vbb[:].rearrange("p h d -> p (h d)"), vbr[:])
        wp_sb = consts.tile([128, 2, D], bf16, tag="wp_sb")
        nc.sync.dma_start(wp_sb[:], wp_in[:])
        wq_sb = consts.tile([128, 8, HL * DH], bf16, tag="wq_sb")
        nc.sync.dma_start(wq_sb[:], wq_in[:])
        wk_sb = consts.tile([128, 8, HL * DH], bf16, tag="wk_sb")
        nc.sync.dma_start(wk_sb[:], wk_in[:])
        wv_sb = consts.tile([128, 8, HL * DH], bf16, tag="wv_sb")
        nc.sync.dma_start(wv_sb[:], wv_in[:])

        # ---- persistent activation tiles ----
        xn_all = persist.tile([128, 16, D], bf16, tag="bigA")   # LN1(x), t-major
        xnT = persist.tile([128, 8, T], bf16, tag="bigB")       # LN1(x)^T
        qT = mid.tile([128, 2, T], bf16, tag="mid")             # q^T (4 heads)
        kT = mid.tile([128, 2, T], bf16, tag="mid")             # k^T
        v_sb = mid.tile([128, 16, HL, DH + 1], bf16, tag="mid")  # v + ones col
        oaT = mid.tile([128, 2, T], bf16, tag="mid")            # attn out^T

        # ================= Phase 1: LN1 + DMA transpose =================
        with ExitStack() as ph:
            work = ph.enter_context(tc.tile_pool(name="p1work", bufs=3))
            small = ph.enter_context(tc.tile_pool(name="p1small", bufs=4))
            for to in range(16):
                xt = work.tile([128, D], f32, tag="xt")
                nc.sync.dma_start(xt[:], x_in[to * 128:(to + 1) * 128, :])
                rstd, nmr = _ln_stats(nc, small, xt, D, epst[:])
                nc.scalar.activation(xn_all[:, to, :], xt[:], AF.Identity,
                                     bias=nmr[:], scale=rstd[:])
                if to >= 4:
                    tt = to - 4
                    nc.sync.dma_start_transpose(
                        xnT[:, :, tt * 128:(tt + 1) * 128], xn_all[:, tt, :])
            for tt in range(12, 16):
                nc.sync.dma_start_transpose(
                    xnT[:, :, tt * 128:(tt + 1) * 128], xn_all[:, tt, :])

        # ========= Phase 2-4: windowed QKV + attention + projection =========
        partial_dram = dram.tile([T, D], bf16)
        rs_out = dram.tile([TG, D], bf16)
        last_prt = None
        with ExitStack() as ph:
            epool = ph.enter_context(tc.tile_pool(name="p3e", bufs=36))
            spool = ph.enter_context(tc.tile_pool(name="p3s", bufs=4))
            work = ph.enter_context(tc.tile_pool(name="p4work", bufs=3))
            ps_q = ph.enter_context(tc.tile_pool(name="psq", bufs=2, space="PSUM"))
            ps_s = ph.enter_context(tc.tile_pool(name="pss", bufs=3, space="PSUM"))
            ps_o = ph.enter_context(tc.tile_pool(name="pso", bufs=2, space="PSUM"))
            ps_p = ph.enter_context(tc.tile_pool(name="psp", bufs=1, space="PSUM"))
            for w in range(4):
                # q^T and k^T for this 512-column window
                for dst, w_sb, bias in ((qT, wq_sb, qbt), (kT, wk_sb, kbt)):
                    for mo in range(2):
                        pq = ps_q.tile([128, 512], f32, tag="pq")
                        for ko in range(8):
                            nc.tensor.matmul(
                                pq[:],
                                w_sb[:, ko, mo * 128:(mo + 1) * 128],
                                xnT[:, ko, w * 512:(w + 1) * 512],
                                start=(ko == 0), stop=(ko == 7))
                        nc.vector.tensor_scalar(
                            dst[:, mo, w * 512:(w + 1) * 512], pq[:],
                            bias[:, mo:mo + 1], None, ALU.add)
                # v rows for this window's four 128-row chunks
                for to in range(4 * w, 4 * w + 4):
                    pv = ps_q.tile([128, 512], f32, tag="pq")
                    for ko in range(8):
                        nc.tensor.matmul(pv[:, 0:256],
                                         xnT[:, ko, to * 128:(to + 1) * 128],
                                         wv_sb[:, ko, :],
                                         start=(ko == 0), stop=(ko == 7))
                    nc.vector.tensor_tensor(
                        v_sb[:, to, :, 0:DH],
                        pv[:, 0:256].rearrange("p (h d) -> p h d", h=HL),
                        vbb[:], ALU.add)
                if w == 0:
                    nc.vector.tensor_copy(
                        out=v_sb[:, :, :, DH:DH + 1],
                        in_=nc.const_aps.tensor(1.0, (128, 16, HL, 1), f32))
                # causal attention for jt = w: heads software-pipelined so
                # head h+1's score matmuls fill PE gaps while head h's PV
                # matmuls wait on exp.
                n_s = 4 * w + 4

                def score_block(h, it):
                    po, ch = (h % 2) * 64, h // 2
                    pss = ps_s.tile([128, 512], f32, tag="pss",
                                    name=f"pss{h}_{w}_{it}")
                    nc.tensor.matmul(
                        pss[:],
                        kT[po:po + 64, ch, it * 128:(it + 1) * 128],
                        qT[po:po + 64, ch, w * 512:(w + 1) * 512],
                        start=True, stop=True)
                    et = epool.tile([128, 512], bf16, tag="et",
                                    name=f"et{h}_{w}_{it}")
                    nc.scalar.activation(et[:], pss[:], AF.Exp, scale=0.125)
                    if it >= 4 * w:
                        nc.vector.tensor_tensor(
                            et[:], et[:], masks[:, it - 4 * w, :], ALU.mult)
                    return et

                ets = {h: [] for h in range(HL)}
                for it in range(n_s):
                    ets[0].append(score_block(0, it))
                for h in range(HL):
                    po, ch = (h % 2) * 64, h // 2
                    pvo = ps_o.tile([128, 512], f32, tag="pvo",
                                    name=f"pvo{h}_{w}")
                    for it in range(n_s):
                        nc.tensor.matmul(pvo[0:DH + 1, :],
                                         v_sb[:, it, h, :], ets[h][it][:],
                                         start=(it == 0), stop=(it == n_s - 1))
                        if h + 1 < HL:
                            ets[h + 1].append(score_block(h + 1, it))
                    lrow = spool.tile([1, 512], f32, tag="lrow")
                    nc.vector.tensor_copy(lrow[:], pvo[DH:DH + 1, :])
                    lb = spool.tile([64, 512], f32, tag="lb")
                    nc.gpsimd.partition_broadcast(lb[:], lrow[:])
                    nc.vector.reciprocal_approx_fast(out=lb[:], in_=lb[:])
                    nc.vector.tensor_tensor(
                        oaT[po:po + 64, ch, w * 512:(w + 1) * 512],
                        pvo[0:DH, :], lb[:], ALU.mult)
                # output projection partial rows for this window
                for to in range(4 * w, 4 * w + 4):
                    prt = work.tile([128, D], bf16, tag="prt")
                    for no in range(2):
                        pp = ps_p.tile([128, 512], f32, tag="pp")
                        for ko in range(2):
                            nc.tensor.matmul(
                                pp[:],
                                oaT[:, ko, to * 128:(to + 1) * 128],
                                wp_sb[:, ko, no * 512:(no + 1) * 512],
                                start=(ko == 0), stop=(ko == 1))
                        nc.vector.tensor_copy(
                            out=prt[:, no * 512:(no + 1) * 512], in_=pp[:])
                    nc.sync.dma_start(
                        partial_dram[to * 128:(to + 1) * 128, :], prt[:])
                    last_prt = prt

            # PE warm-keeper chain: throwaway matmuls that execute while the
            # ReduceScatter is in flight, so the PE clock does not drop.
            warm_scratch = dram.tile([128, 512], f32)
            wsb = spool.tile([128, 512], f32, tag="wsb")
            for dd in range(N_WARM):
                pd = ps_q.tile([128, 512], f32, tag="pq", name=f"warm{dd}")
                nc.tensor.matmul(pd[:], last_prt[:, 0:128], last_prt[:, 0:512],
                                 start=True, stop=True)
                if dd == N_WARM - 1:
                    nc.vector.tensor_copy(wsb[:], pd[:])
            nc.sync.dma_start(warm_scratch[:], wsb[:])

        # ================= Phase 5: ReduceScatter =================
        nc.gpsimd.collective_compute(
            "ReduceScatter", ALU.add,
            replica_groups=[[0, 1, 2, 3], [4, 5, 6, 7]],
            ins=[partial_dram[:].opt()],
            outs=[rs_out[:].opt()])

        # ======= Phase 6: residual + LN2 + DMA transpose (512 rows) =======
        x2 = mid.tile([128, 4, D], bf16, tag="mid")
        xn2T = mid.tile([128, 8, TG], bf16, tag="mid")
        with ExitStack() as ph:
            work = ph.enter_context(tc.tile_pool(name="p6work", bufs=2))
            small = ph.enter_context(tc.tile_pool(name="p6small", bufs=4))
            bp_row = work.tile([1, D], f32, tag="brow", bufs=1)
            nc.sync.dma_start(bp_row[:], bp_in[:])
            bpb = work.tile([128, D], f32, tag="bpb", bufs=1)
            nc.gpsimd.partition_broadcast(bpb[:], bp_row[:])
            b2_row = work.tile([1, D], f32, tag="brow2", bufs=1)
            nc.sync.dma_start(b2_row[:], b2_in[:])
            b2b = work.tile([128, D], f32, tag="b2b", bufs=1)
            nc.gpsimd.partition_broadcast(b2b[:], b2_row[:])
            xn2_all = work.tile([128, 4, D], bf16, tag="xn2a", bufs=1)
            for t2 in range(4):
                prt = work.tile([128, D], bf16, tag="prt")
                nc.sync.dma_start(prt[:], rs_out[t2 * 128:(t2 + 1) * 128, :])
                xrt = work.tile([128, D], f32, tag="xrt")
                nc.sync.dma_start(xrt[:], xg_in[t2 * 128:(t2 + 1) * 128, :])
                x2f = work.tile([128, D], f32, tag="x2f")
                nc.vector.tensor_tensor(x2f[:], prt[:], xrt[:], ALU.add)
                nc.vector.tensor_tensor(x2f[:], x2f[:], bpb[:], ALU.add)
                rstd, nmr = _ln_stats(nc, small, x2f[:], D, epst[:])
                nc.scalar.activation(xn2_all[:, t2, :], x2f[:], AF.Identity,
                                     bias=nmr[:], scale=rstd[:])
                # fold b2 into the residual copy for the FFN epilogue
                nc.vector.tensor_tensor(x2[:, t2, :], x2f[:], b2b[:], ALU.add)
            for t2 in range(4):
                nc.sync.dma_start_transpose(
                    xn2T[:, :, t2 * 128:(t2 + 1) * 128], xn2_all[:, t2, :])

        # ================= Phase 7: FFN first matmul =================
        hT = persist.tile([128, 32, TG], bf16, tag="bigA")
        with ExitStack() as ph:
            wpool = ph.enter_context(tc.tile_pool(name="p7w", bufs=8))
            psum = ph.enter_context(tc.tile_pool(name="p7psum", bufs=6,
                                                 space="PSUM"))
            for mo in range(32):
                w1t = wpool.tile([128, 8, 128], bf16, tag="w1t")
                nc.sync.dma_start(w1t[:], w1_in[:, mo])
                ph_ = psum.tile([128, 512], f32, tag="ph")
                for ko in range(8):
                    nc.tensor.matmul(ph_[:], w1t[:, ko, :], xn2T[:, ko, :],
                                     start=(ko == 0), stop=(ko == 7))
                nc.scalar.activation(hT[:, mo, :], ph_[:], AF.Relu,
                                     bias=b1s[:, mo:mo + 1])

        # ============ Phase 8: FFN second matmul + epilogue ============
        with ExitStack() as ph:
            wpool = ph.enter_context(tc.tile_pool(name="p8w", bufs=8))
            work = ph.enter_context(tc.tile_pool(name="p8work", bufs=2))
            psum = ph.enter_context(tc.tile_pool(name="p8psum", bufs=8,
                                                 space="PSUM"))
            py = [psum.tile([128, 512], f32, tag="py", name=f"py{i}")
                  for i in range(8)]
            for ko in range(32):
                w2t = wpool.tile([128, D], bf16, tag="w2t")
                nc.sync.dma_start(w2t[:], w2_in[ko * 128:(ko + 1) * 128, :])
                for m2 in range(4):
                    for no in range(2):
                        nc.tensor.matmul(
                            py[m2 * 2 + no][:],
                            hT[:, ko, m2 * 128:(m2 + 1) * 128],
                            w2t[:, no * 512:(no + 1) * 512],
                            start=(ko == 0), stop=(ko == 31))
            for m2 in range(4):
                osb = work.tile([128, D], f32, tag="osb")
                for no in range(2):
                    nc.vector.tensor_tensor(
                        osb[:, no * 512:(no + 1) * 512],
                        py[m2 * 2 + no][:], x2[:, m2, no * 512:(no + 1) * 512],
                        ALU.add)
                nc.sync.dma_start(out_dram[m2 * 128:(m2 + 1) * 128, :], osb[:])

    nc.compile()
    return nc


def _prep(inputs):
    x = np.asarray(inputs["x"], np.float32)
    wq = np.asarray(inputs["wq"], np.float32)
    wk = np.asarray(inputs["wk"], np.float32)
    wv = np.asarray(inputs["wv"], np.float32)
    wp = np.asarray(inputs["w_proj"], np.float32)
    bp = np.asarray(inputs["b_proj"], np.float32)
    w1 = np.asarray(inputs["w1"], np.float32)
    b1 = np.asarray(inputs["b1"], np.float32)
    w2 = np.asarray(inputs["w2"], np.float32)
    b2 = np.asarray(inputs["b2"], np.float32)
    ln1_g = np.asarray(inputs["ln1_g"], np.float32)
    ln1_b = np.asarray(inputs["ln1_b"], np.float32)
    ln2_g = np.asarray(inputs["ln2_g"], np.float32)
    ln2_b = np.asarray(inputs["ln2_b"], np.float32)

    # fold LN gains into the adjacent weights (host-side)
    w1f = ln2_g[:, None] * w1                     # [1024, 4096]
    b1f = b1 + ln2_b @ w1                         # [4096]
    w1r = np.ascontiguousarray(
        w1f.reshape(8, 128, 32, 128).transpose(1, 2, 0, 3)).astype(BF16)
    w2r = w2.astype(BF16)
    b1r = np.ascontiguousarray(b1f.reshape(32, 128).T)

    in_maps = []
    for c in range(N_CORES):
        b, g = divmod(c, 4)
        h0 = HL * g
        wqc = np.concatenate([wq[h] for h in range(h0, h0 + HL)], axis=1)
        wkc = np.concatenate([wk[h] for h in range(h0, h0 + HL)], axis=1)
        wvc = np.concatenate([wv[h] for h in range(h0, h0 + HL)], axis=1)
        qb = ln1_b @ wqc                          # [256]
        kb = ln1_b @ wkc
        vb = ln1_b @ wvc
        wqf = ln1_g[:, None] * wqc
        wkf = ln1_g[:, None] * wkc
        wvf = ln1_g[:, None] * wvc
        wpc = wp[h0 * DH:(h0 + HL) * DH, :]
        in_maps.append({
            "x": np.ascontiguousarray(x[b]),
            "xg": np.ascontiguousarray(x[b, g * TG:(g + 1) * TG, :]),
            "wq": np.ascontiguousarray(
                wqf.reshape(8, 128, HL * DH).transpose(1, 0, 2)).astype(BF16),
            "wk": np.ascontiguousarray(
                wkf.reshape(8, 128, HL * DH).transpose(1, 0, 2)).astype(BF16),
            "wv": np.ascontiguousarray(
                wvf.reshape(8, 128, HL * DH).transpose(1, 0, 2)).astype(BF16),
            "qb": np.ascontiguousarray(qb.reshape(2, 128).T),
            "kb": np.ascontiguousarray(kb.reshape(2, 128).T),
            "vb": np.ascontiguousarray(vb.reshape(1, HL * DH)),
            "wp": np.ascontiguousarray(
                wpc.reshape(2, 128, D).transpose(1, 0, 2)).astype(BF16),
            "bp": np.ascontiguousarray(bp.reshape(1, D)),
            "w1": w1r,
            "b1": b1r,
            "w2": w2r,
            "b2": np.ascontiguousarray(b2.reshape(1, D)),
        })
    return in_maps


def _make_runner(nc):
    """Build a cached jitted SPMD executor (mirrors bass2jax.run_bass_via_pjrt
    but jits once and is reused across kernel() calls)."""
    import jax
    from jax.experimental.shard_map import shard_map
    from jax.sharding import Mesh, PartitionSpec
    from concourse import bass2jax as b2j

    b2j.install_neuronx_cc_hook()
    partition_name = (nc.partition_id_tensor.name
                      if nc.partition_id_tensor else None)
    in_names, out_names, out_avals, zero_shapes = [], [], [], []
    for alloc in nc.m.functions[0].allocations:
        if not isinstance(alloc, mybir.MemoryLocationSet):
            continue
        name = alloc.memorylocations[0].name
        if alloc.kind == "ExternalInput":
            if name != partition_name:
                in_names.append(name)
        elif alloc.kind == "ExternalOutput":
            shape = tuple(alloc.tensor_shape)
            dtype = mybir.dt.np(alloc.dtype)
            out_names.append(name)
            out_avals.append(jax.core.ShapedArray(shape, dtype))
            zero_shapes.append((shape, dtype))
    n_params = len(in_names)
    n_outs = len(out_avals)
    all_in_names = list(in_names) + list(out_names)
    if partition_name is not None:
        all_in_names.append(partition_name)
    donate = tuple(range(n_params, n_params + n_outs))

    def _body(*args):
        operands = list(args)
        if partition_name is not None:
            operands.append(b2j.partition_id_tensor())
        outs = b2j._bass_exec_p.bind(
            *operands,
            out_avals=tuple(out_avals),
            in_names=tuple(all_in_names),
            out_names=tuple(out_names),
            lowering_input_output_aliases=(),
            sim_require_finite=True,
            sim_require_nnan=True,
            nc=nc,
        )
        return tuple(outs)

    devices = jax.devices()[:N_CORES]
    mesh = Mesh(np.asarray(devices), ("core",))
    in_specs = (PartitionSpec("core"),) * (n_params + n_outs)
    out_specs = (PartitionSpec("core"),) * n_outs
    sharded = jax.jit(
        shard_map(_body, mesh=mesh, in_specs=in_specs, out_specs=out_specs,
                  check_rep=False),
        donate_argnums=donate, keep_unused=True)

    def run(in_maps):
        concat_in = [
            np.concatenate([np.asarray(in_maps[c][name])
                            for c in range(N_CORES)], axis=0)
            for name in in_names
        ]
        concat_zeros = [
            np.zeros((N_CORES * s[0], *s[1:]), dt) for s, dt in zero_shapes
        ]
        out_arrs = sharded(*concat_in, *concat_zeros)
        return [
            {name: np.asarray(out_arrs[i]).reshape(N_CORES,
                                                   *zero_shapes[i][0])[c]
             for i, name in enumerate(out_names)}
            for c in range(N_CORES)
        ]

    return run


_CACHE = {}


def _get_nc():
    if "nc" not in _CACHE:
        _CACHE["nc"] = _build()
    return _CACHE["nc"]


def _get_runner():
    if "run" not in _CACHE:
        _CACHE["run"] = _make_runner(_get_nc())
    return _CACHE["run"]


def kernel(**inputs):
    run = _get_runner()
    in_maps = _prep(inputs)
    res = run(in_maps)
    B = 2
    out = np.empty((B, T, D), np.float32)
    for c in range(N_CORES):
        b, g = divmod(c, 4)
        out[b, g * TG:(g + 1) * TG, :] = res[c]["out"]
    return out


# revision 17
# speedup vs baseline: 1.3870x; 1.0224x over previous
"""Distributed Trainium2 (Bass/Tile) kernel for a pre-LN transformer block.

Reference computation (per batch element):
    xn = LN1(x); q,k,v = per-head projections of xn
    attn = causal-softmax(q k^T / sqrt(dh)) v
    x1 = x + concat_heads(attn) @ w_proj + b_proj
    out = x1 + relu(LN2(x1) @ w1 + b1) @ w2 + b2

Sharding over 8 NeuronCores: core c handles batch b=c//4 and head group
g=c%4 (4 of 16 heads).  Attention + projection partials are head-parallel;
a ReduceScatter(add) over each 4-core group turns the projection partials
into per-core 512-row slices of x1; the FFN then runs sequence-parallel
(512 rows per core) with no further communication.  The host assembles the
8 [512,1024] outputs into the full [2,2048,1024] result.

Implementation notes:
- Matmul operands are bf16 (fast weight loads, fp32 PSUM accumulation).
- LayerNorm gains/biases are folded into the adjacent weight matrices on
  the host, so the device applies only (x-mu)*rstd.
- Activation transposes use the DMA XBAR (bf16), not the PE array.
- The pipeline is window-ordered: for each 512-column window of the
  sequence, QKV projections, attention, and the output projection are
  emitted together so the tensor engine stays busy.
- A chain of throwaway matmuls keeps the PE clock warm during the
  collective.
"""

import numpy as np
import ml_dtypes

import concourse.bass as bass
import concourse.mybir as mybir
import concourse.tile as tile
from contextlib import ExitStack
from concourse import bacc
from concourse.bass_utils import run_bass_kernel_spmd

T = 2048          # sequence length
D = 1024          # embedding dim
H = 16            # total heads
DH = 64           # head dim
HL = 4            # heads per core
TG = 512          # rows per core in the FFN phase
DF = 4096         # FFN hidden dim
EPS = 1e-5
N_CORES = 8
N_WARM = 100      # PE warm-keeper matmuls during the collective

f32 = mybir.dt.float32
bf16 = mybir.dt.bfloat16
AF = mybir.ActivationFunctionType
ALU = mybir.AluOpType
BF16 = ml_dtypes.bfloat16


def _ln_stats(nc, pool, xt, width, eps_ap):
    """Per-partition mean/var over `width` free elements -> (rstd, neg_mu_rstd)."""
    nchunk = width // 512
    bns = pool.tile([128, nchunk, 6], f32, tag="bns")
    for i in range(nchunk):
        nc.vector.bn_stats(bns[:, i, :], xt[:, i * 512:(i + 1) * 512])
    agg = pool.tile([128, 2], f32, tag="agg")
    nc.vector.bn_aggr(agg[:], bns[:].rearrange("p a b -> p (a b)"))
    std = pool.tile([128, 1], f32, tag="std")
    nc.scalar.activation(std[:], agg[:, 1:2], AF.Sqrt, bias=eps_ap)
    rstd = pool.tile([128, 1], f32, tag="rstd")
    nc.vector.reciprocal(rstd[:], std[:])
    nmr = pool.tile([128, 1], f32, tag="nmr")
    nc.vector.tensor_scalar(nmr[:], agg[:, 0:1], rstd[:], -1.0, ALU.mult, ALU.mult)
    return rstd, nmr


def _build():
    nc = bacc.Bacc("TRN2", target_bir_lowering=False, debug=False,
                   num_devices=N_CORES)

    x_in = nc.dram_tensor("x", [T, D], bf16, kind="ExternalInput")
    wq_in = nc.dram_tensor("wq", [128, 8, HL * DH], bf16, kind="ExternalInput")
    wk_in = nc.dram_tensor("wk", [128, 8, HL * DH], bf16, kind="ExternalInput")
    wv_in = nc.dram_tensor("wv", [128, 8, HL * DH], bf16, kind="ExternalInput")
    qb_in = nc.dram_tensor("qb", [128, 2], f32, kind="ExternalInput")
    kb_in = nc.dram_tensor("kb", [128, 2], f32, kind="ExternalInput")
    vb_in = nc.dram_tensor("vb", [1, HL * DH], f32, kind="ExternalInput")
    wp_in = nc.dram_tensor("wp", [128, 8, D], bf16, kind="ExternalInput")
    w1_in = nc.dram_tensor("w1", [128, 32, 8, 128], bf16, kind="ExternalInput")
    b1_in = nc.dram_tensor("b1", [128, 32], f32, kind="ExternalInput")
    w2_in = nc.dram_tensor("w2", [DF, D], bf16, kind="ExternalInput")
    b2_in = nc.dram_tensor("b2", [1, D], f32, kind="ExternalInput")
    xg_in = nc.dram_tensor("xg", [TG, D], bf16, kind="ExternalInput")
    rm_in = nc.dram_tensor("rmask", [128, 2], f32, kind="ExternalInput")
    out_dram = nc.dram_tensor("out", [TG, D], f32, kind="ExternalOutput")

    with tile.TileContext(nc) as tc, ExitStack() as top:
        persist = top.enter_context(tc.tile_pool(name="persist", bufs=1))
        mid = top.enter_context(tc.tile_pool(name="mid", bufs=4))
        consts = top.enter_context(tc.tile_pool(name="consts", bufs=1))
        dram = top.enter_context(tc.tile_pool(name="dram", bufs=1, space="DRAM"))

        # ---- constants ----
        masks = consts.tile([128, HL, 512], bf16, tag="masks")
        nc.vector.memset(masks[:], 1.0)
        for d in range(HL):
            # keep (1.0) where global_t - global_s >= 0, i.e. f - p - 128*d >= 0
            nc.gpsimd.affine_select(
                out=masks[:, d, :], in_=masks[:, d, :],
                compare_op=ALU.is_ge, fill=0.0,
                base=-128 * d, pattern=[[1, 512]], channel_multiplier=-1)
        epst = consts.tile([128, 1], f32, tag="epst")
        nc.vector.memset(epst[:], EPS)
        b1s = consts.tile([128, 32], f32, tag="b1s")
        nc.sync.dma_start(b1s[:], b1_in[:])
        qbt = consts.tile([128, 2], f32, tag="qbt")
        nc.sync.dma_start(qbt[:], qb_in[:])
        kbt = consts.tile([128, 2], f32, tag="kbt")
        nc.sync.dma_start(kbt[:], kb_in[:])
        vbr = consts.tile([1, HL * DH], f32, tag="vbr")
        nc.sync.dma_start(vbr[:], vb_in[:])
        vbb = consts.tile([128, HL, DH], f32, tag="vbb")
        nc.gpsimd.partition_broadcast(vbb[:].rearrange("p h d -> p (h d)"), vbr[:])
        wp_sb = consts.tile([128, 8, D], bf16, tag="wp_sb")
        nc.sync.dma_start(wp_sb[:], wp_in[:])
        wq_sb = consts.tile([128, 8, HL * DH], bf16, tag="wq_sb")
        nc.sync.dma_start(wq_sb[:], wq_in[:])
        wk_sb = consts.tile([128, 8, HL * DH], bf16, tag="wk_sb")
        nc.sync.dma_start(wk_sb[:], wk_in[:])
        wv_sb = consts.tile([128, 8, HL * DH], bf16, tag="wv_sb")
        nc.sync.dma_start(wv_sb[:], wv_in[:])

        # ---- kernel-start PE warm burst: trips the HAM clock gate to 8/8
        # while LN1 (no PE work) runs ----
        with tc.tile_pool(name="warm0", bufs=2, space="PSUM") as wps:
            for dd in range(50):
                pw = wps.tile([128, 256], f32, tag="w0", name=f"w0_{dd}")
                nc.tensor.matmul(pw[:], wq_sb[:, 0, 0:128], wq_sb[:, 0, 0:256],
                                 start=True, stop=True)

        # ---- persistent activation tiles ----
        xn_all = persist.tile([128, 16, D], bf16, tag="bigA")   # LN1(x), t-major
        xnT = persist.tile([128, 8, T], bf16, tag="bigB")       # LN1(x)^T
        qT = mid.tile([128, 2, T], bf16, tag="mid")             # q^T (4 heads)
        kT = mid.tile([128, 2, T], bf16, tag="mid")             # k^T
        v_sb = mid.tile([128, 16, HL, DH + 1], bf16, tag="mid")  # v + ones col
        oaT = mid.tile([128, 2, T], bf16, tag="mid")            # attn out^T

        # ================= Phase 1: LN1 + DMA transpose =================
        with ExitStack() as ph:
            work = ph.enter_context(tc.tile_pool(name="p1work", bufs=3))
            small = ph.enter_context(tc.tile_pool(name="p1small", bufs=4))
            for to in range(16):
                xt = work.tile([128, D], bf16, tag="xt")
                nc.sync.dma_start(xt[:], x_in[to * 128:(to + 1) * 128, :])
                rstd, nmr = _ln_stats(nc, small, xt, D, epst[:])
                nc.scalar.activation(xn_all[:, to, :], xt[:], AF.Identity,
                                     bias=nmr[:], scale=rstd[:])
                if to >= 4:
                    tt = to - 4
                    nc.sync.dma_start_transpose(
                        xnT[:, :, tt * 128:(tt + 1) * 128], xn_all[:, tt, :])
            for tt in range(12, 16):
                nc.sync.dma_start_transpose(
                    xnT[:, :, tt * 128:(tt + 1) * 128], xn_all[:, tt, :])

        # ========= Phase 2-4: windowed QKV + attention =========
        a2a_in = dram.tile([8, 2 * 128, TG], bf16)
        a2a_out = dram.tile([8, 2 * 128, TG], bf16)
        with ExitStack() as ph:
            epool = ph.enter_context(tc.tile_pool(name="p3e", bufs=36))
            spool = ph.enter_context(tc.tile_pool(name="p3s", bufs=4))
            work = ph.enter_context(tc.tile_pool(name="p4work", bufs=3))
            ps_q = ph.enter_context(tc.tile_pool(name="psq", bufs=2, space="PSUM"))
            ps_s = ph.enter_context(tc.tile_pool(name="pss", bufs=3, space="PSUM"))
            ps_o = ph.enter_context(tc.tile_pool(name="pso", bufs=2, space="PSUM"))
            ps_p = ph.enter_context(tc.tile_pool(name="psp", bufs=1, space="PSUM"))
            for w in range(4):
                # q^T and k^T for this 512-column window
                for dst, w_sb, bias in ((qT, wq_sb, qbt), (kT, wk_sb, kbt)):
                    for mo in range(2):
                        pq = ps_q.tile([128, 512], f32, tag="pq")
                        for ko in range(8):
                            nc.tensor.matmul(
                                pq[:],
                                w_sb[:, ko, mo * 128:(mo + 1) * 128],
                                xnT[:, ko, w * 512:(w + 1) * 512],
                                start=(ko == 0), stop=(ko == 7))
                        nc.vector.tensor_scalar(
                            dst[:, mo, w * 512:(w + 1) * 512], pq[:],
                            bias[:, mo:mo + 1], None, ALU.add)
                # v rows for this window's four 128-row chunks
                for to in range(4 * w, 4 * w + 4):
                    pv = ps_q.tile([128, 512], f32, tag="pq")
                    for ko in range(8):
                        nc.tensor.matmul(pv[:, 0:256],
                                         xnT[:, ko, to * 128:(to + 1) * 128],
                                         wv_sb[:, ko, :],
                                         start=(ko == 0), stop=(ko == 7))
                    nc.vector.tensor_tensor(
                        v_sb[:, to, :, 0:DH],
                        pv[:, 0:256].rearrange("p (h d) -> p h d", h=HL),
                        vbb[:], ALU.add)
                if w == 0:
                    nc.vector.tensor_copy(
                        out=v_sb[:, :, :, DH:DH + 1],
                        in_=nc.const_aps.tensor(1.0, (128, 16, HL, 1), f32))
                # causal attention for jt = w: heads software-pipelined so
                # head h+1's score matmuls fill PE gaps while head h's PV
                # matmuls wait on exp.
                n_s = 4 * w + 4

                def score_block(h, it):
                    po, ch = (h % 2) * 64, h // 2
                    pss = ps_s.tile([128, 512], f32, tag="pss",
                                    name=f"pss{h}_{w}_{it}")
                    nc.tensor.matmul(
                        pss[:],
                        kT[po:po + 64, ch, it * 128:(it + 1) * 128],
                        qT[po:po + 64, ch, w * 512:(w + 1) * 512],
                        start=True, stop=True)
                    et = epool.tile([128, 512], bf16, tag="et",
                                    name=f"et{h}_{w}_{it}")
                    nc.scalar.activation(et[:], pss[:], AF.Exp, scale=0.125)
                    if it >= 4 * w:
                        nc.vector.tensor_tensor(
                            et[:], et[:], masks[:, it - 4 * w, :], ALU.mult)
                    return et

                ets = {h: [] for h in range(HL)}
                for it in range(n_s):
                    ets[0].append(score_block(0, it))
                for h in range(HL):
                    po, ch = (h % 2) * 64, h // 2
                    pvo = ps_o.tile([128, 512], f32, tag="pvo",
                                    name=f"pvo{h}_{w}")
                    for it in range(n_s):
                        nc.tensor.matmul(pvo[0:DH + 1, :],
                                         v_sb[:, it, h, :], ets[h][it][:],
                                         start=(it == 0), stop=(it == n_s - 1))
                        if h + 1 < HL:
                            ets[h + 1].append(score_block(h + 1, it))
                    lrow = spool.tile([1, 512], f32, tag="lrow")
                    nc.vector.tensor_copy(lrow[:], pvo[DH:DH + 1, :])
                    lb = spool.tile([64, 512], f32, tag="lb")
                    nc.gpsimd.partition_broadcast(lb[:], lrow[:])
                    nc.vector.reciprocal_approx_fast(out=lb[:], in_=lb[:])
                    nc.vector.tensor_tensor(
                        oaT[po:po + 64, ch, w * 512:(w + 1) * 512],
                        pvo[0:DH, :], lb[:], ALU.mult)
                # ship this window's head block to its owner rank in both
                # groups (the receiver keeps only its own group's pieces)
                nc.sync.dma_start(
                    a2a_in[w].rearrange("(c p) t -> p c t", p=128),
                    oaT[:, :, w * 512:(w + 1) * 512])
                nc.sync.dma_start(
                    a2a_in[w + 4].rearrange("(c p) t -> p c t", p=128),
                    oaT[:, :, w * 512:(w + 1) * 512])

            # PE warm-keeper chain: throwaway matmuls that execute while the
            # AllToAll is in flight, so the PE clock does not drop.
            warm_scratch = dram.tile([128, 512], f32)
            wsb = spool.tile([128, 512], f32, tag="wsb")
            for dd in range(N_WARM):
                pd = ps_q.tile([128, 512], f32, tag="pq", name=f"warm{dd}")
                nc.tensor.matmul(pd[:], oaT[:, 1, 0:128], oaT[:, 1, 1536:2048],
                                 start=True, stop=True)
                if dd == N_WARM - 1:
                    nc.vector.tensor_copy(wsb[:], pd[:])
            nc.sync.dma_start(warm_scratch[:], wsb[:])

        # ===== Phase 5: AllToAll (each rank collects all 16 heads for its
        # 512 rows) =====
        nc.gpsimd.collective_compute(
            "AllToAll", ALU.bypass,
            replica_groups=[[0, 1, 2, 3, 4, 5, 6, 7]],
            ins=[a2a_in[:].opt()],
            outs=[a2a_out[:].opt()])

        # = Phase 6: projection of gathered heads + residual + LN2 + transpose =
        x2 = mid.tile([128, 4, D], bf16, tag="mid")
        xn2T = mid.tile([128, 8, TG], bf16, tag="mid")
        with ExitStack() as ph:
            work = ph.enter_context(tc.tile_pool(name="p6work", bufs=2))
            small = ph.enter_context(tc.tile_pool(name="p6small", bufs=4))
            psum = ph.enter_context(tc.tile_pool(name="p6psum", bufs=4,
                                                 space="PSUM"))
            b2_row = work.tile([1, D], f32, tag="brow2", bufs=1)
            nc.sync.dma_start(b2_row[:], b2_in[:])
            b2b = work.tile([128, D], f32, tag="b2b", bufs=1)
            nc.gpsimd.partition_broadcast(b2b[:], b2_row[:])
            oa_lo = work.tile([128, 8, TG], bf16, tag="oalo", bufs=1)
            nc.sync.dma_start(
                oa_lo[:],
                a2a_out[0:4].rearrange("r (c p) t -> p (r c) t", p=128))
            oa_hi = work.tile([128, 8, TG], bf16, tag="oahi", bufs=1)
            nc.sync.dma_start(
                oa_hi[:],
                a2a_out[4:8].rearrange("r (c p) t -> p (r c) t", p=128))
            rmt = work.tile([128, 2], f32, tag="rmt", bufs=1)
            nc.sync.dma_start(rmt[:], rm_in[:])
            oa_sb = work.tile([128, 8, TG], bf16, tag="oasb", bufs=1)
            nc.vector.tensor_scalar_mul(oa_sb[:], oa_lo[:], rmt[:, 0:1])
            nc.vector.scalar_tensor_tensor(
                oa_sb[:], oa_hi[:], rmt[:, 1:2], oa_sb[:], ALU.mult, ALU.add)
            xn2_all = work.tile([128, 4, D], bf16, tag="xn2a", bufs=1)
            for t2 in range(4):
                xrt = work.tile([128, D], bf16, tag="xrt")
                nc.sync.dma_start(xrt[:], xg_in[t2 * 128:(t2 + 1) * 128, :])
                x2f = work.tile([128, D], f32, tag="x2f")
                for no in range(2):
                    pp = psum.tile([128, 512], f32, tag="pp")
                    for ko in range(8):
                        nc.tensor.matmul(
                            pp[:],
                            oa_sb[:, ko, t2 * 128:(t2 + 1) * 128],
                            wp_sb[:, ko, no * 512:(no + 1) * 512],
                            start=(ko == 0), stop=(ko == 7))
                    nc.vector.tensor_tensor(
                        x2f[:, no * 512:(no + 1) * 512], pp[:],
                        xrt[:, no * 512:(no + 1) * 512], ALU.add)
                rstd, nmr = _ln_stats(nc, small, x2f[:], D, epst[:])
                nc.scalar.activation(xn2_all[:, t2, :], x2f[:], AF.Identity,
                                     bias=nmr[:], scale=rstd[:])
                # fold b2 into the residual copy for the FFN epilogue
                nc.vector.tensor_tensor(x2[:, t2, :], x2f[:], b2b[:], ALU.add)
                nc.sync.dma_start_transpose(
                    xn2T[:, :, t2 * 128:(t2 + 1) * 128], xn2_all[:, t2, :])

        # ================= Phase 7: FFN first matmul =================
        hT = persist.tile([128, 32, TG], bf16, tag="bigA")
        with ExitStack() as ph:
            wpool = ph.enter_context(tc.tile_pool(name="p7w", bufs=8))
            psum = ph.enter_context(tc.tile_pool(name="p7psum", bufs=6,
                                                 space="PSUM"))
            for mo in range(32):
                w1t = wpool.tile([128, 8, 128], bf16, tag="w1t")
                nc.sync.dma_start(w1t[:], w1_in[:, mo])
                ph_ = psum.tile([128, 512], f32, tag="ph")
                for ko in range(8):
                    nc.tensor.matmul(ph_[:], w1t[:, ko, :], xn2T[:, ko, :],
                                     start=(ko == 0), stop=(ko == 7))
                nc.scalar.activation(hT[:, mo, :], ph_[:], AF.Relu,
                                     bias=b1s[:, mo:mo + 1])

        # ============ Phase 8: FFN second matmul + epilogue ============
        with ExitStack() as ph:
            wpool = ph.enter_context(tc.tile_pool(name="p8w", bufs=8))
            work = ph.enter_context(tc.tile_pool(name="p8work", bufs=2))
            psum = ph.enter_context(tc.tile_pool(name="p8psum", bufs=8,
                                                 space="PSUM"))
            py = [psum.tile([128, 512], f32, tag="py", name=f"py{i}")
                  for i in range(8)]
            for ko in range(32):
                w2t = wpool.tile([128, D], bf16, tag="w2t")
                nc.sync.dma_start(w2t[:], w2_in[ko * 128:(ko + 1) * 128, :])
                for m2 in range(4):
                    for no in range(2):
                        nc.tensor.matmul(
                            py[m2 * 2 + no][:],
                            hT[:, ko, m2 * 128:(m2 + 1) * 128],
                            w2t[:, no * 512:(no + 1) * 512],
                            start=(ko == 0), stop=(ko == 31))
            for m2 in range(4):
                osb = work.tile([128, D], f32, tag="osb")
                for no in range(2):
                    nc.vector.tensor_tensor(
                        osb[:, no * 512:(no + 1) * 512],
                        py[m2 * 2 + no][:], x2[:, m2, no * 512:(no + 1) * 512],
                        ALU.add)
                nc.sync.dma_start(out_dram[m2 * 128:(m2 + 1) * 128, :], osb[:])

    nc.compile()
    return nc


def _prep(inputs):
    x = np.asarray(inputs["x"], np.float32)
    wq = np.asarray(inputs["wq"], np.float32)
    wk = np.asarray(inputs["wk"], np.float32)
    wv = np.asarray(inputs["wv"], np.float32)
    wp = np.asarray(inputs["w_proj"], np.float32)
    bp = np.asarray(inputs["b_proj"], np.float32)
    w1 = np.asarray(inputs["w1"], np.float32)
    b1 = np.asarray(inputs["b1"], np.float32)
    w2 = np.asarray(inputs["w2"], np.float32)
    b2 = np.asarray(inputs["b2"], np.float32)
    ln1_g = np.asarray(inputs["ln1_g"], np.float32)
    ln1_b = np.asarray(inputs["ln1_b"], np.float32)
    ln2_g = np.asarray(inputs["ln2_g"], np.float32)
    ln2_b = np.asarray(inputs["ln2_b"], np.float32)

    # fold LN gains into the adjacent weights (host-side)
    w1f = ln2_g[:, None] * w1                     # [1024, 4096]
    b1f = b1 + ln2_b @ w1                         # [4096]
    w1r = np.ascontiguousarray(
        w1f.reshape(8, 128, 32, 128).transpose(1, 2, 0, 3)).astype(BF16)
    wpr = np.ascontiguousarray(
        wp.reshape(8, 128, D).transpose(1, 0, 2)).astype(BF16)
    w2r = w2.astype(BF16)
    b1r = np.ascontiguousarray(b1f.reshape(32, 128).T)

    in_maps = []
    for c in range(N_CORES):
        b, g = divmod(c, 4)
        h0 = HL * g
        wqc = np.concatenate([wq[h] for h in range(h0, h0 + HL)], axis=1)
        wkc = np.concatenate([wk[h] for h in range(h0, h0 + HL)], axis=1)
        wvc = np.concatenate([wv[h] for h in range(h0, h0 + HL)], axis=1)
        qb = ln1_b @ wqc                          # [256]
        kb = ln1_b @ wkc
        vb = ln1_b @ wvc
        wqf = ln1_g[:, None] * wqc
        wkf = ln1_g[:, None] * wkc
        wvf = ln1_g[:, None] * wvc

        in_maps.append({
            "x": np.ascontiguousarray(x[b]).astype(BF16),
            "xg": np.ascontiguousarray(x[b, g * TG:(g + 1) * TG, :]
                                       + bp[None, :]).astype(BF16),
            "wq": np.ascontiguousarray(
                wqf.reshape(8, 128, HL * DH).transpose(1, 0, 2)).astype(BF16),
            "wk": np.ascontiguousarray(
                wkf.reshape(8, 128, HL * DH).transpose(1, 0, 2)).astype(BF16),
            "wv": np.ascontiguousarray(
                wvf.reshape(8, 128, HL * DH).transpose(1, 0, 2)).astype(BF16),
            "qb": np.ascontiguousarray(qb.reshape(2, 128).T),
            "kb": np.ascontiguousarray(kb.reshape(2, 128).T),
            "vb": np.ascontiguousarray(vb.reshape(1, HL * DH)),
            "wp": wpr,
            "rmask": np.ascontiguousarray(
                np.tile(np.array([[1.0 - (c // 4), float(c // 4)]],
                                 np.float32), (128, 1))),
            "w1": w1r,
            "b1": b1r,
            "w2": w2r,
            "b2": np.ascontiguousarray(b2.reshape(1, D)),
        })
    return in_maps


def _make_runner(nc):
    """Build a cached jitted SPMD executor (mirrors bass2jax.run_bass_via_pjrt
    but jits once and is reused across kernel() calls)."""
    import jax
    from jax.experimental.shard_map import shard_map
    from jax.sharding import Mesh, PartitionSpec
    from concourse import bass2jax as b2j

    b2j.install_neuronx_cc_hook()
    partition_name = (nc.partition_id_tensor.name
                      if nc.partition_id_tensor else None)
    in_names, out_names, out_avals, zero_shapes = [], [], [], []
    for alloc in nc.m.functions[0].allocations:
        if not isinstance(alloc, mybir.MemoryLocationSet):
            continue
        name = alloc.memorylocations[0].name
        if alloc.kind == "ExternalInput":
            if name != partition_name:
                in_names.append(name)
        elif alloc.kind == "ExternalOutput":
            shape = tuple(alloc.tensor_shape)
            dtype = mybir.dt.np(alloc.dtype)
            out_names.append(name)
            out_avals.append(jax.core.ShapedArray(shape, dtype))
            zero_shapes.append((shape, dtype))
    n_params = len(in_names)
    n_outs = len(out_avals)
    all_in_names = list(in_names) + list(out_names)
    if partition_name is not None:
        all_in_names.append(partition_name)
    donate = tuple(range(n_params, n_params + n_outs))

    def _body(*args):
        operands = list(args)
        if partition_name is not None:
            operands.append(b2j.partition_id_tensor())
        outs = b2j._bass_exec_p.bind(
            *operands,
            out_avals=tuple(out_avals),
            in_names=tuple(all_in_names),
            out_names=tuple(out_names),
            lowering_input_output_aliases=(),
            sim_require_finite=True,
            sim_require_nnan=True,
            nc=nc,
        )
        return tuple(outs)

    devices = jax.devices()[:N_CORES]
    mesh = Mesh(np.asarray(devices), ("core",))
    in_specs = (PartitionSpec("core"),) * (n_params + n_outs)
    out_specs = (PartitionSpec("core"),) * n_outs
    sharded = jax.jit(
        shard_map(_body, mesh=mesh, in_specs=in_specs, out_specs=out_specs,
                  check_rep=False),
        donate_argnums=donate, keep_unused=True)

    def run(in_maps):
        concat_in = [
            np.concatenate([np.asarray(in_maps[c][name])
                            for c in range(N_CORES)], axis=0)
            for name in in_names
        ]
        concat_zeros = [
            np.zeros((N_CORES * s[0], *s[1:]), dt) for s, dt in zero_shapes
        ]
        out_arrs = sharded(*concat_in, *concat_zeros)
        return [
            {name: np.asarray(out_arrs[i]).reshape(N_CORES,
                                                   *zero_shapes[i][0])[c]
             for i, name in enumerate(out_names)}
            for c in range(N_CORES)
        ]

    return run


_CACHE = {}


def _get_nc():
    if "nc" not in _CACHE:
        _CACHE["nc"] = _build()
    return _CACHE["nc"]


def _get_runner():
    if "run" not in _CACHE:
        _CACHE["run"] = _make_runner(_get_nc())
    return _CACHE["run"]


def kernel(**inputs):
    run = _get_runner()
    in_maps = _prep(inputs)
    res = run(in_maps)
    B = 2
    out = np.empty((B, T, D), np.float32)
    for c in range(N_CORES):
        b, g = divmod(c, 4)
        out[b, g * TG:(g + 1) * TG, :] = res[c]["out"]
    return out
